# revision 1
# baseline (speedup 1.0000x reference)
"""Trainium2 Bass/Tile kernel for nn_EncoderLayer (dense transformer block).

Strategy: pure data-parallel over batch (B=8 -> 1 batch element per core,
no collectives). Per core, activations are kept feature-major ([D, T]) so
every matmul contracts over the partition axis with weights loaded in their
natural [D_in, D_out] layout; attention P@V consumes row-major V with an
appended mask column that yields the softmax normalizer for free; the FFN2
output is produced row-major (via ffT as the stationary operand) so the
final LayerNorm reduces along the free axis and the result DMAs out
contiguously. Residuals are folded into matmul accumulation chains with an
identity stationary/moving operand. All matmuls run as float32r (full-rate
fp32 PE mode).
"""

import json
import sys

if "/opt/trn_rl_repo" not in sys.path:
    sys.path.insert(0, "/opt/trn_rl_repo")

import numpy as np

import concourse.bass as bass
import concourse.mybir as mybir
import concourse.tile as tile

B, T, CC, DM, H, DH, DFF, K = 8, 1024, 256, 1024, 16, 64, 5120, 3
EMB = CC + DM  # 1280
EPS = 1e-6
f32 = mybir.dt.float32
f32r = mybir.dt.float32r
AF = mybir.ActivationFunctionType
OP = mybir.AluOpType

NT = T // 128          # 8 time tiles
NKE = EMB // 128       # 10 embed k-tiles
NKD = DM // 128        # 8 d_model k-tiles
NMF = DFF // 128       # 40 d_ff tiles
HV = DH + 1            # 65: per-head V columns + mask column


def _mm(nc, out, lhsT, rhs, start, stop):
    nc.tensor.matmul(out, lhsT.bitcast(f32r), rhs.bitcast(f32r), start=start, stop=stop)


def build_nc(phase=99):
    import os
    phase = int(os.environ.get("KPHASE", phase))
    nc = bass.Bass()

    xt_d = nc.declare_dram_parameter("xt", [EMB, T], f32, isOutput=False)
    maskf_d = nc.declare_dram_parameter("maskf", [T], f32, isOutput=False)
    seqf_d = nc.declare_dram_parameter("seqf", [T], f32, isOutput=False)
    convw_d = nc.declare_dram_parameter("convw", [K], f32, isOutput=False)
    wqr_d = nc.declare_dram_parameter("wqr", [8, 8, 128, 128], f32, isOutput=False)
    wkr_d = nc.declare_dram_parameter("wkr", [8, 8, 128, 128], f32, isOutput=False)
    wv_d = nc.declare_dram_parameter("wv", [DM, DM], f32, isOutput=False)
    wor_d = nc.declare_dram_parameter("wor", [8, 8, 128, 128], f32, isOutput=False)
    w1r_d = nc.declare_dram_parameter("w1r", [40, 10, 128, 128], f32, isOutput=False)
    w2_d = nc.declare_dram_parameter("w2", [DFF, EMB], f32, isOutput=False)
    bq_d = nc.declare_dram_parameter("bq", [DM], f32, isOutput=False)
    bk_d = nc.declare_dram_parameter("bk", [DM], f32, isOutput=False)
    bv_d = nc.declare_dram_parameter("bv", [DM], f32, isOutput=False)
    bo_d = nc.declare_dram_parameter("bo", [DM], f32, isOutput=False)
    b1_d = nc.declare_dram_parameter("b1", [DFF], f32, isOutput=False)
    b2_d = nc.declare_dram_parameter("b2", [EMB], f32, isOutput=False)
    g1_d = nc.declare_dram_parameter("g1", [EMB], f32, isOutput=False)
    beta1_d = nc.declare_dram_parameter("beta1", [EMB], f32, isOutput=False)
    g2_d = nc.declare_dram_parameter("g2", [EMB], f32, isOutput=False)
    beta2_d = nc.declare_dram_parameter("beta2", [EMB], f32, isOutput=False)
    onescol_d = nc.declare_dram_parameter("onescol", [128, 1], f32, isOutput=False)
    onesrow_d = nc.declare_dram_parameter("onesrow", [1, 128], f32, isOutput=False)
    ident_d = nc.declare_dram_parameter("ident", [128, 128], f32, isOutput=False)
    bvf_d = nc.declare_dram_parameter("bvf", [128, DM], f32, isOutput=False)
    cwbc_d = nc.declare_dram_parameter("cwbc", [128, K], f32, isOutput=False)
    b2f_d = nc.declare_dram_parameter("b2f", [128, EMB], f32, isOutput=False)
    g2f_d = nc.declare_dram_parameter("g2f", [128, EMB], f32, isOutput=False)
    beta2f_d = nc.declare_dram_parameter("beta2f", [128, EMB], f32, isOutput=False)
    out_d = nc.declare_dram_parameter("out", [T, EMB], f32, isOutput=True)

    h1t_d = nc.dram_tensor("h1t", [EMB, T], f32)

    with tile.TileContext(nc) as tc:
        constp = tc.alloc_tile_pool(name="constp", bufs=1)

        ones_col = constp.tile([128, 1], f32r)
        nc.sync.dma_start(ones_col[:], onescol_d[:].bitcast(f32r))
        ones_row = constp.tile([1, 128], f32r)
        nc.sync.dma_start(ones_row[:], onesrow_d[:].bitcast(f32r))
        ident = constp.tile([128, 128], f32r)
        nc.sync.dma_start(ident[:], ident_d[:].bitcast(f32r))
        epsP = constp.tile([128, 1], f32)
        nc.gpsimd.memset(epsP[:], EPS)

        bqP = constp.tile([128, 8], f32)
        nc.sync.dma_start(bqP[:], bq_d.rearrange("(t p) -> p t", p=128))
        bkP = constp.tile([128, 8], f32)
        nc.sync.dma_start(bkP[:], bk_d.rearrange("(t p) -> p t", p=128))
        boP = constp.tile([128, 8], f32)
        nc.sync.dma_start(boP[:], bo_d.rearrange("(t p) -> p t", p=128))
        b1P = constp.tile([128, 40], f32)
        nc.sync.dma_start(b1P[:], b1_d.rearrange("(t p) -> p t", p=128))
        g1P = constp.tile([128, 10], f32)
        nc.sync.dma_start(g1P[:], g1_d.rearrange("(t p) -> p t", p=128))
        beta1P = constp.tile([128, 10], f32)
        nc.sync.dma_start(beta1P[:], beta1_d.rearrange("(t p) -> p t", p=128))
        maskP = constp.tile([128, 8], f32)
        nc.sync.dma_start(maskP[:], maskf_d.rearrange("(t p) -> p t", p=128))
        seqP = constp.tile([128, 8], f32)
        nc.sync.dma_start(seqP[:], seqf_d.rearrange("(t p) -> p t", p=128))
        seq_row = constp.tile([1, T], f32)
        nc.sync.dma_start(seq_row[:], seqf_d.rearrange("(a t) -> a t", a=1))
        # pre-broadcast constants shipped from host
        bvF = constp.tile([128, DM], f32)
        nc.sync.dma_start(bvF[:], bvf_d[:])
        convw_bc = constp.tile([128, K], f32)
        nc.sync.dma_start(convw_bc[:], cwbc_d[:])

        # ---------------- persistent activations ----------------
        xtp = tc.alloc_tile_pool(name="xtp", bufs=1)
        xt = xtp.tile([128, NKE, T], f32r)
        for k in range(NKE):
            nc.sync.dma_start(xt[:, k, :],
                              xt_d[k * 128:(k + 1) * 128, :].bitcast(f32r))

        attp = tc.alloc_tile_pool(name="attp", bufs=1)
        attT = attp.tile([128, NKD, T], f32r)

        qkvp = tc.alloc_tile_pool(name="qkvp", bufs=1)
        vaug = qkvp.tile([128, NT, H * HV], f32r)
        qt = qkvp.tile([128, NKD, T], f32r)
        kt = qkvp.tile([128, NKD, T], f32r)

        # ---------------- V projection (row-major, masked, augmented) ----
        with (
            tc.tile_pool(name="wvp", bufs=3) as wvp,
            tc.tile_pool(name="vps", bufs=8, space="PSUM") as vps,
            tc.tile_pool(name="vtmp", bufs=3) as vtmp,
        ):
            for n in range(2):
                pss = [vps.tile([128, 512], f32, name=f"vps{i}", tag="vps") for i in range(NT)]
                for k in range(NKD):
                    wvt = wvp.tile([128, 512], f32r)
                    nc.sync.dma_start(
                        wvt[:],
                        wv_d[k * 128:(k + 1) * 128, n * 512:(n + 1) * 512].bitcast(f32r),
                    )
                    for i in range(NT):
                        _mm(nc, pss[i][:], xt[:, 2 + k, i * 128:(i + 1) * 128],
                            wvt[:], k == 0, k == NKD - 1)
                for i in range(NT):
                    tmp = vtmp.tile([128, 512], f32)
                    nc.vector.tensor_add(tmp[:], pss[i][:], bvF[:, n * 512:(n + 1) * 512])
                    dest = vaug[:, i, :].rearrange("p (h c) -> p h c", c=HV)
                    dest = dest[:, n * 8:(n + 1) * 8, 0:DH]
                    nc.vector.tensor_scalar_mul(dest, tmp[:], maskP[:, i:i + 1])
            # mask columns (col 64 of each head slot)
            for i in range(NT):
                mcols = vaug[:, i, :].rearrange("p (h c) -> p h c", c=HV)[:, :, DH:HV]
                mcols = mcols.rearrange("p h c -> p (h c)")
                nc.vector.tensor_copy(mcols, maskP[:, i:i + 1].to_broadcast([128, H]))

        # ---------------- Q/K projections (feature-major) ----------------
        if phase >= 2:
         with (
             tc.tile_pool(name="wqp", bufs=3) as wqp,
             tc.tile_pool(name="qps", bufs=4, space="PSUM") as qps,
         ):
             for wdram, dst, biasP in ((wqr_d, qt, bqP), (wkr_d, kt, bkP)):
                 for m in range(8):
                     wt = wqp.tile([128, 8, 128], f32r, tag="wt")
                     nc.sync.dma_start(wt[:], wdram[m].rearrange("k p q -> p k q").bitcast(f32r))
                     ps = qps.tile([128, 2, 512], f32)
                     for n in range(2):
                         for k in range(NKD):
                             _mm(nc, ps[:, n, :], wt[:, k, :],
                                 xt[:, 2 + k, n * 512:(n + 1) * 512], k == 0, k == NKD - 1)
                     nc.scalar.activation(
                         dst[:, m, :], ps.rearrange("p a b -> p (a b)"), AF.Identity,
                         bias=biasP[:, m:m + 1],
                     )

        # ---------------- attention (two heads interleaved) ----------------
        if phase >= 3:
          with (
              tc.tile_pool(name="upool", bufs=3) as upool,
              tc.tile_pool(name="normp", bufs=2) as normp,
              tc.tile_pool(name="sps", bufs=2, space="PSUM") as spsp,
              tc.tile_pool(name="aps", bufs=1, space="PSUM") as apsp,
          ):
            NH = H if phase >= 4 else 2
            norm_tiles = {}

            def finalize(pair):
                # broadcast 1/norm along partitions via PE outer, scale in place
                for h in pair:
                    prow = (h % 2) * 64
                    ktile = h // 2
                    rps = spsp.tile([64, 2, 512], f32, name="rps", tag="sps")
                    for c in range(2):
                        _mm(nc, rps[:, c, :], ones_row[:, 0:64],
                            norm_tiles[h][:, c * 512:(c + 1) * 512], True, True)
                    dsth = attT[prow:prow + 64, ktile, :]
                    for c in range(2):
                        nc.vector.tensor_mul(
                            dsth[:, c * 512:(c + 1) * 512],
                            dsth[:, c * 512:(c + 1) * 512].bitcast(f32),
                            rps[:, c, :],
                        )

            prev_pair = None
            for hp in range(NH // 2):
                heads = (2 * hp, 2 * hp + 1)
                apss = {}
                for h in heads:
                    apss[h] = apsp.tile([HV, 2, 512], f32,
                                        name=f"aps{h % 2}", tag=f"aps{h % 2}")
                for jt in range(NT):
                    for h in heads:
                        prow = (h % 2) * 64
                        ktile = h // 2
                        sps = spsp.tile([128, 2, 512], f32, name="sps", tag="sps")
                        klhs = kt[prow:prow + 64, ktile, jt * 128:(jt + 1) * 128]
                        for c in range(2):
                            _mm(nc, sps[:, c, :], klhs,
                                qt[prow:prow + 64, ktile, c * 512:(c + 1) * 512],
                                True, True)
                        u = upool.tile([128, T], f32r, name="u", tag="u")
                        nc.scalar.activation(
                            u[:], sps.rearrange("p a b -> p (a b)"), AF.Exp,
                            scale=0.125,
                        )
                        vlhs = vaug[:, jt, h * HV:(h + 1) * HV]
                        for c in range(2):
                            _mm(nc, apss[h][:, c, :], vlhs,
                                u[:, c * 512:(c + 1) * 512], jt == 0, jt == NT - 1)
                for h in heads:
                    prow = (h % 2) * 64
                    ktile = h // 2
                    nt = normp.tile([1, T], f32r, name=f"nt{h % 2}", tag=f"nt{h % 2}")
                    norm_tiles[h] = nt
                    with nc.allow_low_precision(reason="fp32r softmax normalizer"):
                        nc.vector.reciprocal(
                            nt[:],
                            apss[h][DH:HV, :, :].rearrange("p a b -> p (a b)"),
                        )
                    # evict unnormalized attention; normalized in place in finalize()
                    nc.vector.tensor_copy(
                        attT[prow:prow + 64, ktile, :],
                        apss[h][0:DH, :, :].rearrange("p a b -> p (a b)"),
                    )
                if prev_pair is not None:
                    finalize(prev_pair)
                prev_pair = heads
            finalize(prev_pair)

        qkvp.release()

        # ---------------- h1pre = concat(conv, att@wo + bo) + x ----------
        if phase >= 5:
         h1p = tc.alloc_tile_pool(name="h1p", bufs=1)
         h1pre = h1p.tile([128, NKE, T], f32r)

         with tc.tile_pool(name="convp", bufs=2) as convp:
             for kb in range(2):
                 pad = convp.tile([128, T + 2], f32)
                 nc.gpsimd.memset(pad[:, 0:1], 0.0)
                 nc.gpsimd.memset(pad[:, T + 1:T + 2], 0.0)
                 nc.vector.tensor_copy(pad[:, 1:T + 1], xt[:, kb, :].bitcast(f32))
                 a1 = convp.tile([128, T], f32, tag="a1")
                 nc.vector.tensor_scalar_mul(a1[:], pad[:, 0:T], convw_bc[:, 0:1])
                 a2 = convp.tile([128, T], f32, tag="a2")
                 nc.vector.scalar_tensor_tensor(
                     a2[:], pad[:, 1:T + 1], convw_bc[:, 1:2], a1[:], OP.mult, OP.add
                 )
                 a3 = convp.tile([128, T], f32, tag="a3")
                 nc.vector.scalar_tensor_tensor(
                     a3[:], pad[:, 2:T + 2], convw_bc[:, 2:3], a2[:], OP.mult, OP.add
                 )
                 nc.vector.tensor_add(h1pre[:, kb, :], a3[:], xt[:, kb, :].bitcast(f32))

         with (
             tc.tile_pool(name="wop", bufs=3) as wop,
             tc.tile_pool(name="ops", bufs=4, space="PSUM") as opsp,
         ):
             for m in range(8):
                 wt = wop.tile([128, 8, 128], f32r, tag="wo")
                 nc.sync.dma_start(wt[:], wor_d[m].rearrange("k p q -> p k q").bitcast(f32r))
                 for n in range(2):
                     ps = opsp.tile([128, 512], f32)
                     # residual1: I.T @ x-tile seeds the accumulator
                     _mm(nc, ps[:], ident[:], xt[:, 2 + m, n * 512:(n + 1) * 512],
                         True, False)
                     for k in range(NKD):
                         _mm(nc, ps[:], wt[:, k, :],
                             attT[:, k, n * 512:(n + 1) * 512], False, k == NKD - 1)
                     nc.scalar.activation(
                         h1pre[:, 2 + m, n * 512:(n + 1) * 512], ps[:], AF.Identity,
                         bias=boP[:, m:m + 1],
                     )

         # ---------------- LayerNorm 1 (feature axis = partitions) -------
         with (
             tc.tile_pool(name="sqp", bufs=3) as sqp,
             tc.tile_pool(name="vecp", bufs=1) as vecp,
             tc.tile_pool(name="lnps", bufs=1, space="PSUM") as lnps,
             tc.tile_pool(name="lnops", bufs=2, space="PSUM") as lnops,
         ):
             musum = lnps.tile([1, 2, 512], f32, tag="musum")
             sqsum = lnps.tile([1, 2, 512], f32, tag="sqsum")
             for k in range(NKE):
                 sq = sqp.tile([128, T], f32r)
                 nc.vector.tensor_mul(sq[:], h1pre[:, k, :], h1pre[:, k, :])
                 for c in range(2):
                     _mm(nc, musum[:, c, :], ones_col[:],
                         h1pre[:, k, c * 512:(c + 1) * 512], k == 0, k == NKE - 1)
                     _mm(nc, sqsum[:, c, :], ones_col[:],
                         sq[:, c * 512:(c + 1) * 512], k == 0, k == NKE - 1)
             mu = vecp.tile([1, T], f32r)
             nc.vector.tensor_scalar_mul(
                 mu[:], musum.rearrange("p a b -> p (a b)"), 1.0 / EMB
             )
             ex2 = vecp.tile([1, T], f32)
             nc.vector.tensor_scalar_mul(
                 ex2[:], sqsum.rearrange("p a b -> p (a b)"), 1.0 / EMB
             )
             var = vecp.tile([1, T], f32)
             nc.vector.tensor_mul(var[:], mu.bitcast(f32)[:], mu.bitcast(f32)[:])
             nc.vector.tensor_sub(var[:], ex2[:], var[:])
             sd = vecp.tile([1, T], f32)
             nc.scalar.activation(sd[:], var[:], AF.Sqrt, bias=epsP[0:1, :])
             rs = vecp.tile([1, T], f32r)
             with nc.allow_low_precision(reason="fp32r LN1 inv-std"):
                 nc.vector.reciprocal(rs[:], sd[:])
             nc.vector.tensor_mul(rs[:], rs[:], seq_row.bitcast(f32r)[:])  # fold seq_mask
             muF = vecp.tile([128, T], f32, tag="muF")
             rsF = vecp.tile([128, T], f32, tag="rsF")
             for c in range(2):
                 pmu = lnops.tile([128, 512], f32)
                 _mm(nc, pmu[:], ones_row[:], mu[:, c * 512:(c + 1) * 512], True, True)
                 nc.scalar.activation(muF[:, c * 512:(c + 1) * 512], pmu[:], AF.Copy)
                 prs = lnops.tile([128, 512], f32)
                 _mm(nc, prs[:], ones_row[:], rs[:, c * 512:(c + 1) * 512], True, True)
                 nc.scalar.activation(rsF[:, c * 512:(c + 1) * 512], prs[:], AF.Copy)
             for k in range(NKE):
                 t1 = sqp.tile([128, T], f32, tag="t1")
                 nc.vector.tensor_sub(t1[:], h1pre[:, k, :].bitcast(f32), muF[:])
                 t2 = sqp.tile([128, T], f32, tag="t2")
                 nc.vector.tensor_mul(t2[:], t1[:], rsF[:])
                 t3 = sqp.tile([128, T], f32r, tag="t3")
                 nc.scalar.activation(
                     t3[:], t2[:], AF.Identity,
                     bias=beta1P[:, k:k + 1], scale=g1P[:, k:k + 1],
                 )
                 nc.sync.dma_start(h1t_d[k * 128:(k + 1) * 128, :].bitcast(f32r), t3[:])

         h1p.release()
        attp.release()
        xtp.release()

        # ---------------- FFN + LayerNorm 2, in two T-halves -------------
        if phase < 6:
            with tc.tile_pool(name="dummy", bufs=1) as dum:
                z = dum.tile([128, EMB], f32)
                nc.gpsimd.memset(z[:], 0.0)
                for t in range(NT):
                    nc.sync.dma_start(out_d[t * 128:(t + 1) * 128, :], z[:])
            constp.release()
            return nc
        ffnc = tc.alloc_tile_pool(name="ffnc", bufs=1)
        b2F = ffnc.tile([128, EMB], f32)
        nc.sync.dma_start(b2F[:], b2f_d[:])
        g2F = ffnc.tile([128, EMB], f32)
        nc.sync.dma_start(g2F[:], g2f_d[:])
        beta2F = ffnc.tile([128, EMB], f32)
        nc.sync.dma_start(beta2F[:], beta2f_d[:])

        ffp = tc.alloc_tile_pool(name="ffp", bufs=1)
        h1full = ffp.tile([128, NKE, T], f32r)
        for k in range(NKE):
            nc.sync.dma_start(h1full[:, k, :],
                              h1t_d[k * 128:(k + 1) * 128, :].bitcast(f32r))
        out2acc = ffp.tile([128, NT, EMB], f32)
        NSL = ((0, 512), (512, 512), (1024, 256))
        with (
            tc.tile_pool(name="w1p", bufs=3) as w1p,
            tc.tile_pool(name="w2p", bufs=6) as w2p,
            tc.tile_pool(name="blkp", bufs=2) as blkp,
            tc.tile_pool(name="ps1", bufs=2, space="PSUM") as ps1,
            tc.tile_pool(name="ps2", bufs=3, space="PSUM") as ps2,
            tc.tile_pool(name="ln2p", bufs=1) as ln2p,
        ):
            for blk in range(10):
                ffb = blkp.tile([128, 4, T], f32r, tag="ffb")
                for mi in range(4):
                    m = blk * 4 + mi
                    w1t = w1p.tile([128, 10, 128], f32r, tag="w1t")
                    nc.sync.dma_start(w1t[:], w1r_d[m].rearrange("k p q -> p k q").bitcast(f32r))
                    ps = ps1.tile([128, 2, 512], f32)
                    for k in range(NKE):
                        for c in range(2):
                            _mm(nc, ps[:, c, :], w1t[:, k, :],
                                h1full[:, k, c * 512:(c + 1) * 512],
                                k == 0, k == NKE - 1)
                    nc.scalar.activation(
                        ffb[:, mi, :], ps.rearrange("p a b -> p (a b)"),
                        AF.Relu, bias=b1P[:, m:m + 1],
                    )
                w2ts = []
                for ki in range(4):
                    k = blk * 4 + ki
                    w2t = w2p.tile([128, EMB], f32r, name=f"w2t{ki}", tag="w2t")
                    nc.sync.dma_start(
                        w2t[:], w2_d[k * 128:(k + 1) * 128, :].bitcast(f32r)
                    )
                    w2ts.append(w2t)
                for t in range(NT):
                    for n, (nbase, nsz) in enumerate(NSL):
                        pso = ps2.tile([128, 512], f32, name="pso", tag="pso")
                        for ki in range(4):
                            _mm(nc, pso[:, 0:nsz],
                                ffb[:, ki, t * 128:(t + 1) * 128],
                                w2ts[ki][:, nbase:nbase + nsz],
                                ki == 0, ki == 3)
                            if blk == 0 and ki == 0:
                                # residual2 via identity moving operand
                                for kb in range(nbase // 128, (nbase + nsz) // 128):
                                    _mm(nc, pso[:, kb * 128 - nbase:kb * 128 - nbase + 128],
                                        h1full[:, kb, t * 128:(t + 1) * 128], ident[:],
                                        False, False)
                        dst = out2acc[:, t, nbase:nbase + nsz]
                        if blk == 0:
                            nc.vector.tensor_add(dst, pso[:, 0:nsz],
                                                 b2F[:, nbase:nbase + nsz])
                        else:
                            nc.vector.tensor_add(dst, dst, pso[:, 0:nsz])
            # LayerNorm 2 (row-major per time tile) + store
            for gt in range(NT):
                o = out2acc[:, gt, :]
                rsum = ln2p.tile([128, 1], f32, tag="rsum")
                nc.vector.reduce_sum(rsum[:], o, axis=mybir.AxisListType.X)
                muv = ln2p.tile([128, 1], f32, tag="muv")
                nc.vector.tensor_scalar_mul(muv[:], rsum[:], 1.0 / EMB)
                cen = ln2p.tile([128, EMB], f32, tag="cen")
                nc.vector.tensor_scalar_sub(cen[:], o, muv[:])
                sqv = ln2p.tile([128, EMB], f32, tag="sqv")
                nc.vector.tensor_mul(sqv[:], cen[:], cen[:])
                vv = ln2p.tile([128, 1], f32, tag="vv")
                nc.vector.reduce_sum(vv[:], sqv[:], axis=mybir.AxisListType.X)
                nc.vector.tensor_scalar_mul(vv[:], vv[:], 1.0 / EMB)
                sdv = ln2p.tile([128, 1], f32, tag="sdv")
                nc.scalar.activation(sdv[:], vv[:], AF.Sqrt, bias=epsP[:])
                rv = ln2p.tile([128, 1], f32, tag="rv")
                nc.vector.reciprocal(rv[:], sdv[:])
                nc.vector.tensor_mul(rv[:], rv[:], seqP[:, gt:gt + 1])
                t5 = ln2p.tile([128, EMB], f32, tag="t5")
                nc.vector.scalar_tensor_tensor(
                    t5[:], cen[:], rv[:], g2F[:], OP.mult, OP.mult
                )
                t6 = ln2p.tile([128, EMB], f32, tag="sqv2")
                nc.vector.tensor_add(t6[:], t5[:], beta2F[:])
                nc.sync.dma_start(out_d[gt * 128:(gt + 1) * 128, :], t6[:])
        ffp.release()
        ffnc.release()
        constp.release()

    return nc


def _split_matmul_waits(bj: bytes) -> bytes:
    """Walrus codegen allows only one sync-wait on Matmult/DMACopy
    instructions; hoist extra waits onto a preceding EventSemaphore."""
    d = json.loads(bj)
    n = 0
    for f in d["functions"]:
        for blk in f["blocks"]:
            out = []
            for inst in blk["instructions"]:
                si = inst.get("sync_info")
                if (si and si.get("on_wait") and len(si["on_wait"]) >= 2
                        and inst.get("opcode") != "EventSemaphore"):
                    waits = si["on_wait"]
                    for w in waits[:-1]:
                        out.append({
                            "debug": inst.get("debug"),
                            "engine": inst["engine"],
                            "ins": [],
                            "outs": [],
                            "name": f"waitfix_{n}",
                            "opcode": "EventSemaphore",
                            "sync_info": {"on_update": [], "on_wait": [w]},
                        })
                        n += 1
                    si["on_wait"] = waits[-1:]
                out.append(inst)
            blk["instructions"] = out
    return json.dumps(d).encode()


_NC_CACHE = None


def _get_nc():
    global _NC_CACHE
    if _NC_CACHE is None:
        nc = build_nc()
        orig = nc.to_json_bytes
        nc.to_json_bytes = lambda: _split_matmul_waits(orig())
        _NC_CACHE = nc
    return _NC_CACHE


def _prep_core_inputs(x_b, mask_b, seq_b, conv_w, wq, bq, wk, bk, wv, bv, wo, bo,
                      w1, b1, w2, b2, g1, beta1, g2, beta2):
    f = np.float32
    return {
        "xt": np.ascontiguousarray(x_b.T, dtype=f),
        "maskf": np.ascontiguousarray((mask_b == 0).astype(f)),
        "seqf": np.ascontiguousarray(seq_b.astype(f)),
        "convw": np.ascontiguousarray(conv_w.reshape(K).astype(f)),
        "wqr": np.ascontiguousarray(wq.reshape(8, 128, 8, 128).transpose(2, 0, 1, 3)),
        "wkr": np.ascontiguousarray(wk.reshape(8, 128, 8, 128).transpose(2, 0, 1, 3)),
        "wv": np.ascontiguousarray(wv.astype(f)),
        "wor": np.ascontiguousarray(wo.reshape(8, 128, 8, 128).transpose(2, 0, 1, 3)),
        "w1r": np.ascontiguousarray(w1.reshape(10, 128, 40, 128).transpose(2, 0, 1, 3)),
        "w2": np.ascontiguousarray(w2.astype(f)),
        "onescol": np.ones((128, 1), f),
        "onesrow": np.ones((1, 128), f),
        "ident": np.eye(128, dtype=f),
        "bvf": np.ascontiguousarray(np.tile(bv.astype(f)[None, :], (128, 1))),
        "cwbc": np.ascontiguousarray(np.tile(conv_w.reshape(K).astype(f)[None, :], (128, 1))),
        "b2f": np.ascontiguousarray(np.tile(b2.astype(f)[None, :], (128, 1))),
        "g2f": np.ascontiguousarray(np.tile(g2.astype(f)[None, :], (128, 1))),
        "beta2f": np.ascontiguousarray(np.tile(beta2.astype(f)[None, :], (128, 1))),
        "bq": np.ascontiguousarray(bq.astype(f)),
        "bk": np.ascontiguousarray(bk.astype(f)),
        "bv": np.ascontiguousarray(bv.astype(f)),
        "bo": np.ascontiguousarray(bo.astype(f)),
        "b1": np.ascontiguousarray(b1.astype(f)),
        "b2": np.ascontiguousarray(b2.astype(f)),
        "g1": np.ascontiguousarray(g1.astype(f)),
        "beta1": np.ascontiguousarray(beta1.astype(f)),
        "g2": np.ascontiguousarray(g2.astype(f)),
        "beta2": np.ascontiguousarray(beta2.astype(f)),
    }


def kernel(x, att_mask, seq_mask, conv_w, wq, bq, wk, bk, wv, bv, wo, bo,
           w1, b1, w2, b2, g1, beta1, g2, beta2, _trace=False):
    from concourse.bass_utils import run_bass_kernel_spmd

    nc = _get_nc()
    x = np.asarray(x, dtype=np.float32)
    in_maps = []
    for b in range(B):
        in_maps.append(_prep_core_inputs(
            x[b], np.asarray(att_mask)[b], np.asarray(seq_mask)[b, :, 0],
            np.asarray(conv_w), np.asarray(wq), np.asarray(bq), np.asarray(wk),
            np.asarray(bk), np.asarray(wv), np.asarray(bv), np.asarray(wo),
            np.asarray(bo), np.asarray(w1), np.asarray(b1), np.asarray(w2),
            np.asarray(b2), np.asarray(g1), np.asarray(beta1), np.asarray(g2),
            np.asarray(beta2)))
    res = run_bass_kernel_spmd(nc, in_maps, list(range(B)), trace=_trace)
    out = np.stack([res.results[i]["out"] for i in range(B)], axis=0)
    if _trace:
        return out, res
    return out



# revision 4
# speedup vs baseline: 1.0023x; 1.0023x over previous
"""Trainium2 Bass/Tile kernel for nn_EncoderLayer (dense transformer block).

Data-parallel over batch (B=8 -> 1 element/core, no collectives). v2:
- All matmuls in fp16 (full PE rate, ~0.02% rel err) with f32 PSUM; LN
  stats and residual paths in f32.
- Attention: feature-major Q/K/V; softmax via exp(s/8 - 3) with the shift
  cancelling against an appended mask-column normalizer; the 1/norm
  broadcast is rebuilt inside the retired PV accumulator bank so attention
  fits exactly in 8 PSUM banks; normalized heads are written back into the
  dead kt slices (attT aliases kt's storage).
- LN1 stays in SBUF; biases/residuals fold into PSUM seeds (rank-1
  ones-outer-products) or STT evictions; conv branch runs in-place on
  Pool/DVE.
- FFN runs in two k-rounds (FFN1 half -> FFN2 half) so the ff activations
  and the w2 half fit SBUF together; round 0 seeds b2 + the h1 residual
  (fp16 identity-matmul transposes) and parks partials in fp16; round 1
  adds the second half and runs LN2 per time tile, streaming out.
"""

import json
import sys

if "/opt/trn_rl_repo" not in sys.path:
    sys.path.insert(0, "/opt/trn_rl_repo")

import numpy as np

import concourse.bass as bass
import concourse.mybir as mybir
import concourse.tile as tile

B, T, CC, DM, H, DH, DFF, K = 8, 1024, 256, 1024, 16, 64, 5120, 3
EMB = CC + DM  # 1280
EPS = 1e-6
f32 = mybir.dt.float32
f32r = mybir.dt.float32r
f16 = mybir.dt.float16
AF = mybir.ActivationFunctionType
OP = mybir.AluOpType

NT = T // 128          # 8 time tiles
NKE = EMB // 128       # 10 embed k-tiles
NKD = DM // 128        # 8 d_model k-tiles
HV = DH + 1            # 65 = V dims + mask column
F16 = np.float16

# column offsets in the packed [128, NCONST] f32 const blob
_C = {}
_o = 0
for _name, _w in (("bqP", 8), ("bkP", 8), ("boP", 8), ("maskP", 8),
                  ("seqP", 8), ("b1P", 40), ("g1P", 10), ("beta1P", 10),
                  ("g2F", EMB), ("beta2F", EMB), ("onescol", 1)):
    _C[_name] = (_o, _o + _w)
    _o += _w
NCONST = _o
# row-vector blob [1, NROW]
_R = {}
_o = 0
for _name, _w in (("onesrow", 128), ("bvrow", DM), ("b2row", EMB),
                  ("seqrow", T)):
    _R[_name] = (_o, _o + _w)
    _o += _w
NROW = _o


def _mmr(nc, out, lhsT, rhs, start, stop):
    nc.tensor.matmul(out, lhsT.bitcast(f32r), rhs.bitcast(f32r),
                     start=start, stop=stop)


def build_nc(phase=99):
    import os
    phase = int(os.environ.get("KPHASE", phase))
    nc = bass.Bass()

    xt_d = nc.declare_dram_parameter("xt", [128, NKE, T], f32, isOutput=False)
    xh_d = nc.declare_dram_parameter("xh", [128, NKD, T], f16, isOutput=False)
    wqh_d = nc.declare_dram_parameter("wqh", [4, 128, 2, NKD, 128], f16, isOutput=False)
    wkh_d = nc.declare_dram_parameter("wkh", [4, 128, 2, NKD, 128], f16, isOutput=False)
    wvh_d = nc.declare_dram_parameter("wvh", [128, NKD, DM], f16, isOutput=False)
    woh_d = nc.declare_dram_parameter("woh", [4, 128, 2, NKD, 128], f16, isOutput=False)
    w1h_d = nc.declare_dram_parameter("w1h", [20, 128, 2, NKE, 128], f16, isOutput=False)
    w2h_d = nc.declare_dram_parameter("w2h", [2, 128, 20, EMB], f16, isOutput=False)
    consts_d = nc.declare_dram_parameter("consts", [128, NCONST], f32, isOutput=False)
    crow_d = nc.declare_dram_parameter("crow", [1, NROW], f32, isOutput=False)
    identh_d = nc.declare_dram_parameter("identh", [128, 128], f16, isOutput=False)
    cwbc_d = nc.declare_dram_parameter("cwbc", [128, K], f32, isOutput=False)
    onescol_d = nc.declare_dram_parameter("onescol", [128, 1], f32, isOutput=False)
    out_d = nc.declare_dram_parameter("out", [T, EMB], f32, isOutput=True)

    with tile.TileContext(nc) as tc:
        constp = tc.alloc_tile_pool(name="constp", bufs=1)
        cb = constp.tile([128, NCONST], f32)
        cr = constp.tile([1, NROW], f32r)
        identh = constp.tile([128, 128], f16)
        cwbc = constp.tile([128, K], f32)
        epsP = constp.tile([128, 1], f32)
        nc.gpsimd.memset(epsP[:], EPS)
        nthreeP = constp.tile([128, 1], f32)
        nc.gpsimd.memset(nthreeP[:], -3.0)
        onescolP = constp.tile([128, 1], f32r)

        def C(name):
            a, b = _C[name]
            return cb[:, a:b]

        def R(name, lo=None, hi=None):
            a, b = _R[name]
            if lo is not None:
                return cr[:, a + lo:a + hi]
            return cr[:, a:b]

        # ------------- persistent pools (right stack, LIFO by release) ----
        h1prep = tc.alloc_tile_pool(name="h1prep", bufs=1, side="right")
        h1pre = h1prep.tile([128, NKE, T], f32r)
        xtp = tc.alloc_tile_pool(name="xtp", bufs=1, side="right")
        xt = xtp.tile([128, NKE, T], f32)
        qktp = tc.alloc_tile_pool(name="qktp", bufs=1, side="right")
        qt = qktp.tile([128, NKD, T], f16)
        kt = qktp.tile([128, NKD, T], f16)   # attT aliases kt after scores
        vaup = tc.alloc_tile_pool(name="vaup", bufs=1, side="right")
        vaug = vaup.tile([128, NT, H, HV], f16)
        wvp = tc.alloc_tile_pool(name="wvp", bufs=1, side="right")
        wvh = wvp.tile([128, NKD, DM], f16)
        xhp = tc.alloc_tile_pool(name="xhp", bufs=1, side="right")
        xh = xhp.tile([128, NKD, T], f16)

        # xh + the first projection weights gate the PE pipeline; everything
        # else (consts, xt, wv) rides the Act queue or follows on SP.
        nc.sync.dma_start(xh[:, 0:4, :], xh_d[:, 0:4, :])
        nc.sync.dma_start(xh[:, 4:NKD, :], xh_d[:, 4:NKD, :])

        # ------------- Q/K/V projections (fp16) ---------------------------
        with (
            tc.tile_pool(name="wst", bufs=4) as wst,
            tc.tile_pool(name="qkps", bufs=2, space="PSUM") as qkps,
            tc.tile_pool(name="vps", bufs=2, space="PSUM") as vps,
        ):
            # pre-issue the wq stream on SP (it gates the PE pipeline); the
            # big non-urgent loads follow on SP; wk groups are issued on the
            # Act queue as their slot frees up during the Q pass
            wts = []
            for g in range(4):
                wt = wst.tile([128, 2, NKD, 128], f16, tag="w")
                nc.sync.dma_start(wt[:], wqh_d[g])
                wts.append(wt)
            nc.sync.dma_start(cb[:, 0:100], consts_d[:, 0:100])
            nc.sync.dma_start(cr[:], crow_d[:].bitcast(f32r))
            nc.sync.dma_start(cb[:, 100:NCONST], consts_d[:, 100:NCONST])
            nc.sync.dma_start(xt[:, 0:2, :], xt_d[:, 0:2, :])
            nc.sync.dma_start(cwbc[:], cwbc_d[:])
            nc.sync.dma_start(onescolP[:], onescol_d[:].bitcast(f32r))
            nc.sync.dma_start(xt[:, 2:NKE, :], xt_d[:, 2:NKE, :])
            nc.sync.dma_start(wvh[:], wvh_d[:])
            nc.sync.dma_start(identh[:], identh_d[:])

            # conv branch (in-place in h1pre, Pool/DVE):
            # y[t] = w0*x[t-1] + w1*x[t] + w2*x[t+1], zero-padded; then +x.
            for kb, eng in ((0, nc.vector), (1, nc.vector)):
                dst = h1pre[:, kb, :]
                eng.tensor_scalar_mul(dst, xt[:, kb, :], cwbc[:, 1:2])
                eng.scalar_tensor_tensor(
                    dst[:, 0:T - 1], xt[:, kb, 1:T], cwbc[:, 2:3],
                    dst[:, 0:T - 1], OP.mult, OP.add)
                eng.scalar_tensor_tensor(
                    dst[:, 1:T], xt[:, kb, 0:T - 1], cwbc[:, 0:1],
                    dst[:, 1:T], OP.mult, OP.add)
                eng.tensor_add(dst, dst, xt[:, kb, :])

            def proj_group(wt, dst, bias, g):
                for mi in range(2):
                    m = 2 * g + mi
                    ps = qkps.tile([128, 2, 512], f32, tag="qk")
                    for c in range(2):
                        for k in range(NKD):
                            nc.tensor.matmul(
                                ps[:, c, :], wt[:, mi, k],
                                xh[:, k, c * 512:(c + 1) * 512],
                                start=(k == 0), stop=(k == NKD - 1))
                    nc.scalar.activation(
                        dst[:, m, :], ps.rearrange("p a b -> p (a b)"),
                        AF.Identity, bias=C(bias)[:, m:m + 1])

            wkts = []
            for g in range(4):
                proj_group(wts[g], qt, "bqP", g)
                wkt = wst.tile([128, 2, NKD, 128], f16, tag="w")
                nc.scalar.dma_start(wkt[:], wkh_d[g])
                wkts.append(wkt)
            for g in range(4):
                proj_group(wkts[g], kt, "bkP", g)
            for i in range(NT):
                for n in range(2):
                    ps = vps.tile([128, 512], f32, tag="v")
                    _mmr(nc, ps[:], R("onesrow", 0, 128),
                         R("bvrow", n * 512, (n + 1) * 512), True, False)
                    for k in range(NKD):
                        nc.tensor.matmul(
                            ps[:], xh[:, k, i * 128:(i + 1) * 128],
                            wvh[:, k, n * 512:(n + 1) * 512],
                            start=False, stop=(k == NKD - 1))
                    dest = vaug[:, i, n * 8:(n + 1) * 8, 0:DH]
                    nc.scalar.activation(
                        dest, ps.rearrange("p (h c) -> p h c", c=DH),
                        AF.Identity, scale=C("maskP")[:, i:i + 1])
                mcols = vaug[:, i, :, DH:DH + 1].rearrange("p h c -> p (h c)")
                nc.vector.tensor_copy(
                    mcols, C("maskP")[:, i:i + 1].to_broadcast([128, H]))
        xhp.release()
        wvp.release()

        # ------------- attention ------------------------------------------
        if phase >= 2:
            with (
                tc.tile_pool(name="spsp", bufs=2, space="PSUM") as spsp,
                tc.tile_pool(name="apsp", bufs=2, space="PSUM") as apsp,
                tc.tile_pool(name="u2p", bufs=3) as u2p,
                tc.tile_pool(name="finp", bufs=1) as finp,
            ):
                for h in range(H):
                    ktile, prow = h // 2, (h % 2) * 64
                    aps = apsp.tile([HV, 2, 512], f32, tag="aps")
                    for jt in range(NT):
                        sps = spsp.tile([128, 2, 512], f32, tag="sps")
                        for c in range(2):
                            nc.tensor.matmul(
                                sps[:, c, :],
                                kt[prow:prow + 64, ktile, jt * 128:(jt + 1) * 128],
                                qt[prow:prow + 64, ktile, c * 512:(c + 1) * 512],
                                start=True, stop=True)
                        u2t = u2p.tile([128, T], f16, tag="u2")
                        # exp(s/8 - 3): the shift cancels against the
                        # mask-column normalizer; keeps u in fp16 range
                        nc.scalar.activation(
                            u2t[:], sps.rearrange("p a b -> p (a b)"),
                            AF.Exp, scale=0.125, bias=nthreeP[:])
                        for c in range(2):
                            nc.tensor.matmul(
                                aps[:, c, :], vaug[:, jt, h, :],
                                u2t[:, c * 512:(c + 1) * 512],
                                start=(jt == 0), stop=(jt == NT - 1))
                    # finalize: 1/norm, copy out the unnormalized head,
                    # broadcast 1/norm into the retired aps bank, then
                    # scale-evict into the dead kt slice (attT alias)
                    nt_ = finp.tile([1, T], f32r, tag=f"nt{h % 2}", name=f"nt{h % 2}")
                    with nc.allow_low_precision(reason="softmax normalizer"):
                        nc.vector.reciprocal(
                            nt_[:], aps[DH:HV, :, :].rearrange("p a b -> p (a b)"))
                    ab = finp.tile([64, T], f16, tag=f"ab{h % 2}", name=f"ab{h % 2}")
                    nc.vector.tensor_copy(
                        ab[:], aps[0:DH, :, :].rearrange("p a b -> p (a b)"))
                    for c in range(2):
                        _mmr(nc, aps[0:DH, c, :], R("onesrow", 0, DH),
                             nt_[:, c * 512:(c + 1) * 512], True, True)
                    nc.vector.tensor_mul(
                        kt[prow:prow + 64, ktile, :], ab[:],
                        aps[0:DH, :, :].rearrange("p a b -> p (a b)"))
            vaup.release()

        # ------------- out-proj + LN1 -------------------------------------
        if phase >= 3:
            h1bp = tc.alloc_tile_pool(name="h1bp", bufs=1)
            h1b = h1bp.tile([128, NKE, T], f16)
            # preload the sqrt activation table off the critical path (the
            # attention exps are done; everything later lives in the
            # sqrt_and_others table)
            scr1 = h1bp.tile([128, 1], f32)
            nc.scalar.activation(scr1[:], epsP[:], AF.Sqrt, bias=epsP[:])
            with (
                tc.tile_pool(name="wost", bufs=3) as wost,
                tc.tile_pool(name="ops", bufs=4, space="PSUM") as opsp,
                tc.tile_pool(name="lnps", bufs=1, space="PSUM") as lnps,
                tc.tile_pool(name="sqp", bufs=2) as sqp,
            ):
                musum = lnps.tile([1, 2, 512], f32, tag="musum")
                sqsum = lnps.tile([1, 2, 512], f32, tag="sqsum")

                def stats(k):
                    for c in range(2):
                        cs = slice(c * 512, (c + 1) * 512)
                        sq = sqp.tile([128, 512], f32r, tag="sq")
                        nc.vector.tensor_mul(sq[:], h1pre[:, k, cs], h1pre[:, k, cs])
                        _mmr(nc, musum[:, c, :], onescolP[:],
                             h1pre[:, k, cs], k == 0, k == NKE - 1)
                        _mmr(nc, sqsum[:, c, :], onescolP[:],
                             sq[:], k == 0, k == NKE - 1)

                stats(0)
                stats(1)
                for g in range(4):
                    wt = wost.tile([128, 2, NKD, 128], f16, tag="wo")
                    nc.scalar.dma_start(wt[:], woh_d[g])
                    for mi in range(2):
                        m = 2 * g + mi
                        for c in range(2):
                            cs = slice(c * 512, (c + 1) * 512)
                            ps = opsp.tile([128, 512], f32, tag="o")
                            for k in range(NKD):
                                nc.tensor.matmul(
                                    ps[:], wt[:, mi, k], kt[:, k, cs],
                                    start=(k == 0), stop=(k == NKD - 1))
                            nc.vector.scalar_tensor_tensor(
                                h1pre[:, 2 + m, cs], ps[:],
                                C("boP")[:, m:m + 1],
                                xt[:, 2 + m, cs], OP.add, OP.add)
                            del ps
                        # stats lag one m-tile so the PE never waits on the
                        # DVE eviction of the tile it is summing
                        if m >= 1:
                            stats(1 + m)
                stats(9)

                # ---------------- LN1 scalars + broadcasts ----------------
                with tc.tile_pool(name="lnvp", bufs=1) as lnvp:
                    mu = lnvp.tile([1, T], f32r)
                    nc.vector.tensor_scalar_mul(
                        mu[:], musum.rearrange("p a b -> p (a b)"), 1.0 / EMB)
                    ex2 = lnvp.tile([1, T], f32r)
                    nc.vector.tensor_scalar_mul(
                        ex2[:], sqsum.rearrange("p a b -> p (a b)"), 1.0 / EMB)
                    sd = lnvp.tile([1, T], f32r)
                    nc.vector.tensor_mul(sd[:], mu[:], mu[:])
                    nc.vector.tensor_sub(ex2[:], ex2[:], sd[:])
                    nc.scalar.activation(sd[:], ex2[:], AF.Sqrt, bias=epsP[0:1, :])
                    rs = ex2  # reuse (dead after the Sqrt read)
                    with nc.allow_low_precision(reason="LN1 inv-std"):
                        nc.vector.reciprocal(rs[:], sd[:])
                    nc.vector.tensor_mul(rs[:], rs[:], R("seqrow"))
                    muF = lnvp.tile([128, T], f16)
                    rsF = lnvp.tile([128, T], f16)
                    for c in range(2):
                        cs = slice(c * 512, (c + 1) * 512)
                        pb = opsp.tile([128, 512], f32, tag="o")
                        _mmr(nc, pb[:], R("onesrow", 0, 128), mu[:, cs], True, True)
                        nc.scalar.activation(muF[:, cs], pb[:], AF.Copy)
                        pb2 = opsp.tile([128, 512], f32, tag="o")
                        _mmr(nc, pb2[:], R("onesrow", 0, 128), rs[:, cs], True, True)
                        nc.scalar.activation(rsF[:, cs], pb2[:], AF.Copy)

                    # ------------- LN1 normalize, c-half major -----------
                    for c in range(2):
                        cs = slice(c * 512, (c + 1) * 512)
                        for k in range(NKE):
                            eng = nc.vector if k % 2 == 0 else nc.gpsimd
                            t1 = sqp.tile([128, 512], f32, tag=f"t1{k % 2}",
                                          name=f"t1{k % 2}")
                            eng.tensor_sub(t1[:], h1pre[:, k, cs], muF[:, cs])
                            t2 = sqp.tile([128, 512], f32, tag=f"t2{k % 2}",
                                          name=f"t2{k % 2}")
                            eng.tensor_mul(t2[:], t1[:], rsF[:, cs])
                            nc.scalar.activation(
                                h1b[:, k, cs], t2[:], AF.Identity,
                                bias=C("beta1P")[:, k:k + 1],
                                scale=C("g1P")[:, k:k + 1])

        if phase < 4:
            with tc.tile_pool(name="dummy", bufs=1) as dum:
                z = dum.tile([128, EMB], f32)
                nc.gpsimd.memset(z[:], 0.0)
                for t in range(NT):
                    nc.sync.dma_start(out_d[t * 128:(t + 1) * 128, :], z[:])
            constp.release()
            return nc

        qktp.release()
        xtp.release()
        h1prep.release()

        # ------------- FFN in two k-rounds + LN2 --------------------------
        accp = tc.alloc_tile_pool(name="accp", bufs=1)
        acc = accp.tile([128, NT, EMB], f16)
        ffhp = tc.alloc_tile_pool(name="ffhp", bufs=1)
        NSL = ((0, 512), (512, 512), (1024, 256))
        # w1st allocated below w2hp so the streamed w1 tiles do not overlap
        # the (still-live) h1pre region and get WAR-gated behind LN1
        with (
            tc.tile_pool(name="w1st", bufs=3) as w1st,
            tc.tile_pool(name="w2hp", bufs=1) as w2hp,
            tc.tile_pool(name="ps1", bufs=2, space="PSUM") as ps1p,
            tc.tile_pool(name="ps2", bufs=1, space="PSUM") as ps2p,
            tc.tile_pool(name="o2a", bufs=2) as o2a,
            tc.tile_pool(name="o2p", bufs=1) as o2p,
            tc.tile_pool(name="ln2p", bufs=2) as ln2p,
        ):
          for rnd in range(2):
            ffh = ffhp.tile([128, 20, T], f16, tag="ffh")
            w2t = w2hp.tile([128, 20, EMB], f16, tag="w2t")
            if True:
                for g in range(10):
                    w1t = w1st.tile([128, 2, NKE, 128], f16, tag="w1")
                    nc.sync.dma_start(w1t[:], w1h_d[10 * rnd + g])
                    if g == 2:
                        # w2 half in chunks behind the first w1 tiles: keeps
                        # the DMA pipe busy without head-of-line blocking
                        for cch in range(4):
                            nc.sync.dma_start(
                                w2t[:, 5 * cch:5 * cch + 5, :],
                                w2h_d[rnd, :, 5 * cch:5 * cch + 5, :])
                    for mi in range(2):
                        ml = 2 * g + mi
                        m = 20 * rnd + ml
                        ps = ps1p.tile([128, 2, 512], f32, tag="f1")
                        for c in range(2):
                            for k in range(NKE):
                                nc.tensor.matmul(
                                    ps[:, c, :], w1t[:, mi, k],
                                    h1b[:, k, c * 512:(c + 1) * 512],
                                    start=(k == 0), stop=(k == NKE - 1))
                        nc.scalar.activation(
                            ffh[:, ml, :], ps.rearrange("p a b -> p (a b)"),
                            AF.Relu, bias=C("b1P")[:, m:m + 1])
            if True:
                for t in range(NT):
                    ts = slice(t * 128, (t + 1) * 128)
                    if rnd == 1:
                        out2 = o2a.tile([128, EMB], f32, tag="out2")
                        rsums = [ln2p.tile([128, 1], f32, tag=f"rs{n}",
                                           name=f"rs{n}") for n in range(3)]
                    psos = []
                    for n, (nb, nsz) in enumerate(NSL):
                        pso = ps2p.tile([128, nsz], f32, tag=f"pso{n}",
                                        name=f"pso{n}")
                        psos.append(pso)
                        nc.tensor.matmul(pso[:], ffh[:, 0, ts],
                                         w2t[:, 0, nb:nb + nsz],
                                         start=True, stop=False)
                        if rnd == 0:
                            _mmr(nc, pso[:], R("onesrow", 0, 128),
                                 R("b2row", nb, nb + nsz), False, False)
                            for kb in range(nb // 128, (nb + nsz) // 128):
                                nc.tensor.matmul(
                                    pso[:, kb * 128 - nb:kb * 128 - nb + 128],
                                    h1b[:, kb, ts], identh[:],
                                    start=False, stop=False)
                        for kl in range(1, 20):
                            nc.tensor.matmul(pso[:], ffh[:, kl, ts],
                                             w2t[:, kl, nb:nb + nsz],
                                             start=False, stop=(kl == 19))
                        if rnd == 0:
                            nc.scalar.activation(
                                acc[:, t, nb:nb + nsz], pso[:], AF.Identity)
                            continue
                        nc.scalar.activation(
                            out2[:, nb:nb + nsz], pso[:], AF.Identity,
                            accum_out=rsums[n][:])
                    if rnd == 0:
                        continue
                    # round 1: add the parked half, then LN2 + store
                    accsum = ln2p.tile([128, 1], f32, tag="accsum", name="accsum")
                    for n, (nb, nsz) in enumerate(NSL):
                        nc.vector.tensor_add(
                            out2[:, nb:nb + nsz], out2[:, nb:nb + nsz],
                            acc[:, t, nb:nb + nsz])
                    nc.vector.reduce_sum(accsum[:], acc[:, t, :],
                                         axis=mybir.AxisListType.X)
                    nmu = ln2p.tile([128, 1], f32, tag="nmu", name="nmu")
                    nc.vector.tensor_add(nmu[:], rsums[0][:], rsums[1][:])
                    nc.vector.tensor_add(nmu[:], nmu[:], rsums[2][:])
                    nc.vector.tensor_add(nmu[:], nmu[:], accsum[:])
                    nc.vector.tensor_scalar_mul(nmu[:], nmu[:], -1.0 / EMB)
                    cen = o2p.tile([128, EMB], f32, tag="cen")
                    nc.scalar.activation(cen[:], out2[:], AF.Identity, bias=nmu[:])
                    vv = ln2p.tile([128, 1], f32, tag="vv", name="vv")
                    sq2 = o2p.tile([128, EMB], f32, tag="sq2")
                    nc.scalar.activation(sq2[:], cen[:], AF.Square,
                                         accum_out=vv[:])
                    nc.vector.tensor_scalar_mul(vv[:], vv[:], 1.0 / EMB)
                    sdv = ln2p.tile([128, 1], f32, tag="sdv", name="sdv")
                    nc.scalar.activation(sdv[:], vv[:], AF.Sqrt, bias=epsP[:])
                    rv = ln2p.tile([128, 1], f32, tag="rv", name="rv")
                    with nc.allow_low_precision(reason="LN2 inv-std"):
                        nc.vector.reciprocal(rv[:], sdv[:])
                    nc.vector.tensor_mul(rv[:], rv[:], C("seqP")[:, t:t + 1])
                    eng = nc.vector if t % 2 == 1 else nc.gpsimd
                    t5 = o2p.tile([128, EMB], f32, tag="t5")
                    if t % 2 == 1:
                        eng.scalar_tensor_tensor(
                            t5[:], cen[:], rv[:], C("g2F"), OP.mult, OP.mult)
                    else:
                        # Pool cannot run STT on HW: scale on Act, mult on Pool
                        cen2 = o2p.tile([128, EMB], f32, tag="cen2")
                        nc.scalar.activation(cen2[:], cen[:], AF.Identity,
                                             scale=rv[:])
                        eng.tensor_mul(t5[:], cen2[:], C("g2F"))
                    eng.tensor_add(t5[:], t5[:], C("beta2F"))
                    nc.sync.dma_start(out_d[ts, :], t5[:])
        ffhp.release()
        accp.release()
        h1bp.release()
        constp.release()

    return nc


def _split_matmul_waits(bj: bytes) -> bytes:
    """Walrus codegen allows only one sync-wait on Matmult/DMACopy
    instructions; hoist extra waits onto a preceding EventSemaphore."""
    d = json.loads(bj)
    n = 0
    for f in d["functions"]:
        for blk in f["blocks"]:
            out = []
            for inst in blk["instructions"]:
                si = inst.get("sync_info")
                if (si and si.get("on_wait") and len(si["on_wait"]) >= 2
                        and inst.get("opcode") != "EventSemaphore"):
                    waits = si["on_wait"]
                    for w in waits[:-1]:
                        out.append({
                            "debug": inst.get("debug"),
                            "engine": inst["engine"],
                            "ins": [],
                            "outs": [],
                            "name": f"waitfix_{n}",
                            "opcode": "EventSemaphore",
                            "sync_info": {"on_update": [], "on_wait": [w]},
                        })
                        n += 1
                    si["on_wait"] = waits[-1:]
                out.append(inst)
            blk["instructions"] = out
    return json.dumps(d).encode()


_NC_CACHE = None


def _get_nc():
    global _NC_CACHE
    if _NC_CACHE is None:
        nc = build_nc()
        orig = nc.to_json_bytes
        nc.to_json_bytes = lambda: _split_matmul_waits(orig())
        _NC_CACHE = nc
    return _NC_CACHE


def _prep_core_inputs(x_b, mask_b, seq_b, conv_w, wq, bq, wk, bk, wv, bv, wo, bo,
                      w1, b1, w2, b2, g1, beta1, g2, beta2):
    f = np.float32
    x_b = np.asarray(x_b, dtype=f)                      # [T, EMB]
    xt = x_b.T.reshape(NKE, 128, T).transpose(1, 0, 2)  # [128, k, T]
    xh = x_b[:, CC:].T.reshape(NKD, 128, T).transpose(1, 0, 2)

    def wpack(w):  # [DM, DM] -> [g, p, mi, k, q]
        return np.ascontiguousarray(
            w.reshape(NKD, 128, 8, 128).transpose(2, 1, 0, 3)  # [m, p, k, q]
            .reshape(4, 2, 128, NKD, 128).transpose(0, 2, 1, 3, 4).astype(F16))

    wvh = np.ascontiguousarray(
        np.asarray(wv, f).reshape(NKD, 128, DM).transpose(1, 0, 2).astype(F16))
    w1h = np.ascontiguousarray(
        np.asarray(w1, f).reshape(NKE, 128, 40, 128)
        .transpose(2, 1, 0, 3)                       # [m, p, k, q]
        .reshape(20, 2, 128, NKE, 128).transpose(0, 2, 1, 3, 4).astype(F16))
    w2h = np.ascontiguousarray(
        np.asarray(w2, f).reshape(2, 20, 128, EMB).transpose(0, 2, 1, 3).astype(F16))

    # reference: scores = where(att_mask != 0, -1e9, scores) — attended
    # keys are those with att_mask == 0
    maskf = (np.asarray(mask_b) == 0).astype(f)
    consts = np.zeros((128, NCONST), f)

    def setC(name, val):
        a, b = _C[name]
        consts[:, a:b] = val

    setC("bqP", np.asarray(bq, f).reshape(8, 128).T)
    setC("bkP", np.asarray(bk, f).reshape(8, 128).T)
    setC("boP", np.asarray(bo, f).reshape(8, 128).T)
    setC("maskP", maskf.reshape(8, 128).T)
    setC("seqP", np.asarray(seq_b, f).reshape(8, 128).T)
    setC("b1P", np.asarray(b1, f).reshape(40, 128).T)
    setC("g1P", np.asarray(g1, f).reshape(10, 128).T)
    setC("beta1P", np.asarray(beta1, f).reshape(10, 128).T)
    setC("g2F", np.tile(np.asarray(g2, f)[None, :], (128, 1)))
    setC("beta2F", np.tile(np.asarray(beta2, f)[None, :], (128, 1)))
    setC("onescol", 1.0)

    crow = np.zeros((1, NROW), f)

    def setR(name, val):
        a, b = _R[name]
        crow[0, a:b] = val

    setR("onesrow", 1.0)
    setR("bvrow", np.asarray(bv, f))
    setR("b2row", np.asarray(b2, f))
    setR("seqrow", np.asarray(seq_b, f))

    return {
        "xt": np.ascontiguousarray(xt),
        "xh": np.ascontiguousarray(xh.astype(F16)),
        "wqh": wpack(np.asarray(wq, f)),
        "wkh": wpack(np.asarray(wk, f)),
        "wvh": wvh,
        "woh": wpack(np.asarray(wo, f)),
        "w1h": w1h,
        "w2h": w2h,
        "consts": consts,
        "crow": crow,
        "identh": np.eye(128).astype(F16),
        "cwbc": np.tile(np.asarray(conv_w, f).reshape(K)[None, :], (128, 1)),
        "onescol": np.ones((128, 1), f),
    }


def kernel(x, att_mask, seq_mask, conv_w, wq, bq, wk, bk, wv, bv, wo, bo,
           w1, b1, w2, b2, g1, beta1, g2, beta2, _trace=False):
    from concourse.bass_utils import run_bass_kernel_spmd

    nc = _get_nc()
    x = np.asarray(x, dtype=np.float32)
    in_maps = []
    for b in range(B):
        in_maps.append(_prep_core_inputs(
            x[b], np.asarray(att_mask)[b], np.asarray(seq_mask)[b, :, 0],
            np.asarray(conv_w), np.asarray(wq), np.asarray(bq), np.asarray(wk),
            np.asarray(bk), np.asarray(wv), np.asarray(bv), np.asarray(wo),
            np.asarray(bo), np.asarray(w1), np.asarray(b1), np.asarray(w2),
            np.asarray(b2), np.asarray(g1), np.asarray(beta1), np.asarray(g2),
            np.asarray(beta2)))
    res = run_bass_kernel_spmd(nc, in_maps, list(range(B)), trace=_trace)
    out = np.stack([res.results[i]["out"] for i in range(B)], axis=0)
    if _trace:
        return out, res
    return out


# revision 5
# speedup vs baseline: 1.0064x; 1.0041x over previous
"""Trainium2 Bass/Tile kernel for nn_EncoderLayer (dense transformer block).

Data-parallel over batch (B=8 -> 1 element/core, no collectives). v2:
- All matmuls in fp16 (full PE rate, ~0.02% rel err) with f32 PSUM; LN
  stats and residual paths in f32.
- Attention: feature-major Q/K/V; softmax via exp(s/8 - 3) with the shift
  cancelling against an appended mask-column normalizer; the 1/norm
  broadcast is rebuilt inside the retired PV accumulator bank so attention
  fits exactly in 8 PSUM banks; normalized heads are written back into the
  dead kt slices (attT aliases kt's storage).
- LN1 stays in SBUF; biases/residuals fold into PSUM seeds (rank-1
  ones-outer-products) or STT evictions; conv branch runs in-place on
  Pool/DVE.
- FFN runs in two k-rounds (FFN1 half -> FFN2 half) so the ff activations
  and the w2 half fit SBUF together; round 0 seeds b2 + the h1 residual
  (fp16 identity-matmul transposes) and parks partials in fp16; round 1
  adds the second half and runs LN2 per time tile, streaming out.
"""

import json
import sys

if "/opt/trn_rl_repo" not in sys.path:
    sys.path.insert(0, "/opt/trn_rl_repo")

import numpy as np

import concourse.bass as bass
import concourse.mybir as mybir
import concourse.tile as tile

B, T, CC, DM, H, DH, DFF, K = 8, 1024, 256, 1024, 16, 64, 5120, 3
EMB = CC + DM  # 1280
EPS = 1e-6
f32 = mybir.dt.float32
f32r = mybir.dt.float32r
f16 = mybir.dt.float16
AF = mybir.ActivationFunctionType
OP = mybir.AluOpType

NT = T // 128          # 8 time tiles
NKE = EMB // 128       # 10 embed k-tiles
NKD = DM // 128        # 8 d_model k-tiles
HV = DH + 1            # 65 = V dims + mask column
F16 = np.float16

# column offsets in the packed [128, NCONST] f32 const blob
_C = {}
_o = 0
for _name, _w in (("bqP", 8), ("bkP", 8), ("boP", 8), ("maskP", 8),
                  ("seqP", 8), ("b1P", 40), ("g1P", 10), ("beta1P", 10),
                  ("g2F", EMB), ("beta2F", EMB), ("onescol", 1)):
    _C[_name] = (_o, _o + _w)
    _o += _w
NCONST = _o
# row-vector blob [1, NROW]
_R = {}
_o = 0
for _name, _w in (("onesrow", 128), ("bvrow", DM), ("b2row", EMB),
                  ("seqrow", T)):
    _R[_name] = (_o, _o + _w)
    _o += _w
NROW = _o


def _mmr(nc, out, lhsT, rhs, start, stop):
    nc.tensor.matmul(out, lhsT.bitcast(f32r), rhs.bitcast(f32r),
                     start=start, stop=stop)


def build_nc(phase=99):
    import os
    phase = int(os.environ.get("KPHASE", phase))
    nc = bass.Bass()

    xt_d = nc.declare_dram_parameter("xt", [128, NKE, T], f32, isOutput=False)
    xh_d = nc.declare_dram_parameter("xh", [128, NKD, T], f16, isOutput=False)
    wqh_d = nc.declare_dram_parameter("wqh", [4, 128, 2, NKD, 128], f16, isOutput=False)
    wkh_d = nc.declare_dram_parameter("wkh", [4, 128, 2, NKD, 128], f16, isOutput=False)
    wvh_d = nc.declare_dram_parameter("wvh", [128, NKD, DM], f16, isOutput=False)
    woh_d = nc.declare_dram_parameter("woh", [4, 128, 2, NKD, 128], f16, isOutput=False)
    w1h_d = nc.declare_dram_parameter("w1h", [20, 128, 2, NKE, 128], f16, isOutput=False)
    w2h_d = nc.declare_dram_parameter("w2h", [2, 128, 20, EMB], f16, isOutput=False)
    consts_d = nc.declare_dram_parameter("consts", [128, NCONST], f32, isOutput=False)
    crow_d = nc.declare_dram_parameter("crow", [1, NROW], f32, isOutput=False)
    identh_d = nc.declare_dram_parameter("identh", [128, 128], f16, isOutput=False)
    cwbc_d = nc.declare_dram_parameter("cwbc", [128, K], f32, isOutput=False)
    onescol_d = nc.declare_dram_parameter("onescol", [128, 1], f32, isOutput=False)
    out_d = nc.declare_dram_parameter("out", [T, EMB], f32, isOutput=True)

    with tile.TileContext(nc) as tc:
        constp = tc.alloc_tile_pool(name="constp", bufs=1)
        cb = constp.tile([128, NCONST], f32)
        cr = constp.tile([1, NROW], f32r)
        identh = constp.tile([128, 128], f16)
        cwbc = constp.tile([128, K], f32)
        epsP = constp.tile([128, 1], f32)
        nc.gpsimd.memset(epsP[:], EPS)
        nthreeP = constp.tile([128, 1], f32)
        nc.gpsimd.memset(nthreeP[:], -3.0)
        onescolP = constp.tile([128, 1], f32r)

        def C(name):
            a, b = _C[name]
            return cb[:, a:b]

        def R(name, lo=None, hi=None):
            a, b = _R[name]
            if lo is not None:
                return cr[:, a + lo:a + hi]
            return cr[:, a:b]

        # ------------- persistent pools (right stack, LIFO by release) ----
        h1prep = tc.alloc_tile_pool(name="h1prep", bufs=1, side="right")
        h1pre = h1prep.tile([128, NKE, T], f32r)
        xtp = tc.alloc_tile_pool(name="xtp", bufs=1, side="right")
        xt = xtp.tile([128, NKE, T], f32)
        qktp = tc.alloc_tile_pool(name="qktp", bufs=1, side="right")
        qt = qktp.tile([128, NKD, T], f16)
        kt = qktp.tile([128, NKD, T], f16)   # attT aliases kt after scores
        vaup = tc.alloc_tile_pool(name="vaup", bufs=1, side="right")
        vaug = vaup.tile([128, NT, H, HV], f16)
        wvp = tc.alloc_tile_pool(name="wvp", bufs=1, side="right")
        wvh = wvp.tile([128, NKD, DM], f16)
        xhp = tc.alloc_tile_pool(name="xhp", bufs=1, side="right")
        xh = xhp.tile([128, NKD, T], f16)

        # xh + the first projection weights gate the PE pipeline; everything
        # else (consts, xt, wv) rides the Act queue or follows on SP.
        nc.sync.dma_start(xh[:, 0:4, :], xh_d[:, 0:4, :])

        # ------------- Q/K/V projections (fp16) ---------------------------
        with (
            tc.tile_pool(name="wst", bufs=4) as wst,
            tc.tile_pool(name="qkps", bufs=2, space="PSUM") as qkps,
            tc.tile_pool(name="vps", bufs=2, space="PSUM") as vps,
        ):
            # pre-issue the wq stream on SP (it gates the PE pipeline); the
            # big non-urgent loads follow on SP; wk groups are issued on the
            # Act queue as their slot frees up during the Q pass
            wts = []
            for g in range(4):
                wt = wst.tile([128, 2, NKD, 128], f16, tag="w")
                nc.sync.dma_start(wt[:], wqh_d[g])
                wts.append(wt)
                if g == 0:
                    nc.sync.dma_start(xh[:, 4:NKD, :], xh_d[:, 4:NKD, :])
            nc.sync.dma_start(cb[:, 0:100], consts_d[:, 0:100])
            nc.sync.dma_start(cr[:], crow_d[:].bitcast(f32r))
            nc.sync.dma_start(cb[:, 100:NCONST], consts_d[:, 100:NCONST])
            nc.sync.dma_start(xt[:, 0:2, :], xt_d[:, 0:2, :])
            nc.sync.dma_start(cwbc[:], cwbc_d[:])
            nc.sync.dma_start(onescolP[:], onescol_d[:].bitcast(f32r))
            nc.sync.dma_start(xt[:, 2:NKE, :], xt_d[:, 2:NKE, :])
            nc.sync.dma_start(wvh[:], wvh_d[:])
            nc.sync.dma_start(identh[:], identh_d[:])

            # conv branch (in-place in h1pre, Pool/DVE):
            # y[t] = w0*x[t-1] + w1*x[t] + w2*x[t+1], zero-padded; then +x.
            for kb, eng in ((0, nc.vector), (1, nc.vector)):
                dst = h1pre[:, kb, :]
                eng.tensor_scalar_mul(dst, xt[:, kb, :], cwbc[:, 1:2])
                eng.scalar_tensor_tensor(
                    dst[:, 0:T - 1], xt[:, kb, 1:T], cwbc[:, 2:3],
                    dst[:, 0:T - 1], OP.mult, OP.add)
                eng.scalar_tensor_tensor(
                    dst[:, 1:T], xt[:, kb, 0:T - 1], cwbc[:, 0:1],
                    dst[:, 1:T], OP.mult, OP.add)
                eng.tensor_add(dst, dst, xt[:, kb, :])

            def proj_group(wt, dst, bias, g):
                for mi in range(2):
                    m = 2 * g + mi
                    ps = qkps.tile([128, 2, 512], f32, tag="qk")
                    for c in range(2):
                        for k in range(NKD):
                            nc.tensor.matmul(
                                ps[:, c, :], wt[:, mi, k],
                                xh[:, k, c * 512:(c + 1) * 512],
                                start=(k == 0), stop=(k == NKD - 1))
                    nc.scalar.activation(
                        dst[:, m, :], ps.rearrange("p a b -> p (a b)"),
                        AF.Identity, bias=C(bias)[:, m:m + 1])

            wkts = []
            for g in range(4):
                proj_group(wts[g], qt, "bqP", g)
                wkt = wst.tile([128, 2, NKD, 128], f16, tag="w")
                nc.scalar.dma_start(wkt[:], wkh_d[g])
                wkts.append(wkt)
            for g in range(4):
                proj_group(wkts[g], kt, "bkP", g)
            for i in range(NT):
                for n in range(2):
                    ps = vps.tile([128, 512], f32, tag="v")
                    _mmr(nc, ps[:], R("onesrow", 0, 128),
                         R("bvrow", n * 512, (n + 1) * 512), True, False)
                    for k in range(NKD):
                        nc.tensor.matmul(
                            ps[:], xh[:, k, i * 128:(i + 1) * 128],
                            wvh[:, k, n * 512:(n + 1) * 512],
                            start=False, stop=(k == NKD - 1))
                    dest = vaug[:, i, n * 8:(n + 1) * 8, 0:DH]
                    nc.vector.tensor_scalar_mul(
                        dest, ps.rearrange("p (h c) -> p h c", c=DH),
                        C("maskP")[:, i:i + 1])
                mcols = vaug[:, i, :, DH:DH + 1].rearrange("p h c -> p (h c)")
                nc.vector.tensor_copy(
                    mcols, C("maskP")[:, i:i + 1].to_broadcast([128, H]))
        xhp.release()
        wvp.release()

        # ------------- attention ------------------------------------------
        if phase >= 2:
            with (
                tc.tile_pool(name="spsp", bufs=2, space="PSUM") as spsp,
                tc.tile_pool(name="apsp", bufs=2, space="PSUM") as apsp,
                tc.tile_pool(name="u2p", bufs=3) as u2p,
                tc.tile_pool(name="finp", bufs=1) as finp,
            ):
                for h in range(H):
                    ktile, prow = h // 2, (h % 2) * 64
                    aps = apsp.tile([HV, 2, 512], f32, tag="aps")
                    for jt in range(NT):
                        sps = spsp.tile([128, 2, 512], f32, tag="sps")
                        for c in range(2):
                            nc.tensor.matmul(
                                sps[:, c, :],
                                kt[prow:prow + 64, ktile, jt * 128:(jt + 1) * 128],
                                qt[prow:prow + 64, ktile, c * 512:(c + 1) * 512],
                                start=True, stop=True)
                        u2t = u2p.tile([128, T], f16, tag="u2")
                        # exp(s/8 - 3): the shift cancels against the
                        # mask-column normalizer; keeps u in fp16 range
                        nc.scalar.activation(
                            u2t[:], sps.rearrange("p a b -> p (a b)"),
                            AF.Exp, scale=0.125, bias=nthreeP[:])
                        for c in range(2):
                            nc.tensor.matmul(
                                aps[:, c, :], vaug[:, jt, h, :],
                                u2t[:, c * 512:(c + 1) * 512],
                                start=(jt == 0), stop=(jt == NT - 1))
                    # finalize: 1/norm, copy out the unnormalized head,
                    # broadcast 1/norm into the retired aps bank, then
                    # scale-evict into the dead kt slice (attT alias)
                    nt_ = finp.tile([1, T], f32r, tag=f"nt{h % 2}", name=f"nt{h % 2}")
                    with nc.allow_low_precision(reason="softmax normalizer"):
                        nc.vector.reciprocal(
                            nt_[:], aps[DH:HV, :, :].rearrange("p a b -> p (a b)"))
                    ab = finp.tile([64, T], f16, tag=f"ab{h % 2}", name=f"ab{h % 2}")
                    nc.vector.tensor_copy(
                        ab[:], aps[0:DH, :, :].rearrange("p a b -> p (a b)"))
                    for c in range(2):
                        _mmr(nc, aps[0:DH, c, :], R("onesrow", 0, DH),
                             nt_[:, c * 512:(c + 1) * 512], True, True)
                    nc.vector.tensor_mul(
                        kt[prow:prow + 64, ktile, :], ab[:],
                        aps[0:DH, :, :].rearrange("p a b -> p (a b)"))
            vaup.release()

        # ------------- out-proj + LN1 -------------------------------------
        if phase >= 3:
            h1bp = tc.alloc_tile_pool(name="h1bp", bufs=1)
            h1b = h1bp.tile([128, NKE, T], f16)
            # preload the sqrt activation table off the critical path (the
            # attention exps are done; everything later lives in the
            # sqrt_and_others table)
            scr1 = h1bp.tile([128, 1], f32)
            nc.scalar.activation(scr1[:], epsP[:], AF.Sqrt, bias=epsP[:])
            with (
                tc.tile_pool(name="wost", bufs=3) as wost,
                tc.tile_pool(name="ops", bufs=4, space="PSUM") as opsp,
                tc.tile_pool(name="lnps", bufs=1, space="PSUM") as lnps,
                tc.tile_pool(name="sqp", bufs=2) as sqp,
            ):
                musum = lnps.tile([1, 2, 512], f32, tag="musum")
                sqsum = lnps.tile([1, 2, 512], f32, tag="sqsum")

                def stats(k):
                    for c in range(2):
                        cs = slice(c * 512, (c + 1) * 512)
                        sq = sqp.tile([128, 512], f32r, tag="sq")
                        nc.vector.tensor_mul(sq[:], h1pre[:, k, cs], h1pre[:, k, cs])
                        _mmr(nc, musum[:, c, :], onescolP[:],
                             h1pre[:, k, cs], k == 0, k == NKE - 1)
                        _mmr(nc, sqsum[:, c, :], onescolP[:],
                             sq[:], k == 0, k == NKE - 1)

                stats(0)
                stats(1)
                for g in range(4):
                    wt = wost.tile([128, 2, NKD, 128], f16, tag="wo")
                    nc.scalar.dma_start(wt[:], woh_d[g])
                    for mi in range(2):
                        m = 2 * g + mi
                        for c in range(2):
                            cs = slice(c * 512, (c + 1) * 512)
                            ps = opsp.tile([128, 512], f32, tag="o")
                            for k in range(NKD):
                                nc.tensor.matmul(
                                    ps[:], wt[:, mi, k], kt[:, k, cs],
                                    start=(k == 0), stop=(k == NKD - 1))
                            nc.vector.scalar_tensor_tensor(
                                h1pre[:, 2 + m, cs], ps[:],
                                C("boP")[:, m:m + 1],
                                xt[:, 2 + m, cs], OP.add, OP.add)
                            del ps
                        # stats lag one m-tile so the PE never waits on the
                        # DVE eviction of the tile it is summing
                        if m >= 1:
                            stats(1 + m)
                stats(9)

                # ---------------- LN1 scalars + broadcasts ----------------
                with tc.tile_pool(name="lnvp", bufs=1) as lnvp:
                    mu = lnvp.tile([1, T], f32r)
                    nc.vector.tensor_scalar_mul(
                        mu[:], musum.rearrange("p a b -> p (a b)"), 1.0 / EMB)
                    ex2 = lnvp.tile([1, T], f32r)
                    nc.vector.tensor_scalar_mul(
                        ex2[:], sqsum.rearrange("p a b -> p (a b)"), 1.0 / EMB)
                    sd = lnvp.tile([1, T], f32r)
                    nc.vector.tensor_mul(sd[:], mu[:], mu[:])
                    nc.vector.tensor_sub(ex2[:], ex2[:], sd[:])
                    nc.scalar.activation(sd[:], ex2[:], AF.Sqrt, bias=epsP[0:1, :])
                    rs = ex2  # reuse (dead after the Sqrt read)
                    with nc.allow_low_precision(reason="LN1 inv-std"):
                        nc.vector.reciprocal(rs[:], sd[:])
                    nc.vector.tensor_mul(rs[:], rs[:], R("seqrow"))
                    muF = lnvp.tile([128, T], f16)
                    rsF = lnvp.tile([128, T], f16)
                    for c in range(2):
                        cs = slice(c * 512, (c + 1) * 512)
                        pb = opsp.tile([128, 512], f32, tag="o")
                        _mmr(nc, pb[:], R("onesrow", 0, 128), mu[:, cs], True, True)
                        nc.scalar.activation(muF[:, cs], pb[:], AF.Copy)
                        pb2 = opsp.tile([128, 512], f32, tag="o")
                        _mmr(nc, pb2[:], R("onesrow", 0, 128), rs[:, cs], True, True)
                        nc.scalar.activation(rsF[:, cs], pb2[:], AF.Copy)

                    # ------------- LN1 normalize, c-half major -----------
                    for c in range(2):
                        cs = slice(c * 512, (c + 1) * 512)
                        for k in range(NKE):
                            eng = nc.vector if k % 2 == 0 else nc.gpsimd
                            t1 = sqp.tile([128, 512], f32, tag=f"t1{k % 2}",
                                          name=f"t1{k % 2}")
                            eng.tensor_sub(t1[:], h1pre[:, k, cs], muF[:, cs])
                            t2 = sqp.tile([128, 512], f32, tag=f"t2{k % 2}",
                                          name=f"t2{k % 2}")
                            eng.tensor_mul(t2[:], t1[:], rsF[:, cs])
                            nc.scalar.activation(
                                h1b[:, k, cs], t2[:], AF.Identity,
                                bias=C("beta1P")[:, k:k + 1],
                                scale=C("g1P")[:, k:k + 1])

        if phase < 4:
            with tc.tile_pool(name="dummy", bufs=1) as dum:
                z = dum.tile([128, EMB], f32)
                nc.gpsimd.memset(z[:], 0.0)
                for t in range(NT):
                    nc.sync.dma_start(out_d[t * 128:(t + 1) * 128, :], z[:])
            constp.release()
            return nc

        qktp.release()
        xtp.release()
        h1prep.release()

        # ------------- FFN in two k-rounds + LN2 --------------------------
        accp = tc.alloc_tile_pool(name="accp", bufs=1)
        acc = accp.tile([128, NT, EMB], f16)
        ffhp = tc.alloc_tile_pool(name="ffhp", bufs=1)
        NSL = ((0, 512), (512, 512), (1024, 256))
        # w1st allocated below w2hp so the streamed w1 tiles do not overlap
        # the (still-live) h1pre region and get WAR-gated behind LN1
        with (
            tc.tile_pool(name="w1st", bufs=3) as w1st,
            tc.tile_pool(name="w2hp", bufs=1) as w2hp,
            tc.tile_pool(name="ps1", bufs=2, space="PSUM") as ps1p,
            tc.tile_pool(name="ps2", bufs=1, space="PSUM") as ps2p,
            tc.tile_pool(name="o2a", bufs=2) as o2a,
            tc.tile_pool(name="o2p", bufs=1) as o2p,
            tc.tile_pool(name="ln2p", bufs=2) as ln2p,
        ):
          for rnd in range(2):
            ffh = ffhp.tile([128, 20, T], f16, tag="ffh")
            w2t = w2hp.tile([128, 20, EMB], f16, tag="w2t")
            if True:
                for g in range(10):
                    w1t = w1st.tile([128, 2, NKE, 128], f16, tag="w1")
                    nc.sync.dma_start(w1t[:], w1h_d[10 * rnd + g])
                    if g == 2:
                        # w2 half in chunks behind the first w1 tiles: keeps
                        # the DMA pipe busy without head-of-line blocking
                        for cch in range(4):
                            nc.sync.dma_start(
                                w2t[:, 5 * cch:5 * cch + 5, :],
                                w2h_d[rnd, :, 5 * cch:5 * cch + 5, :])
                    for mi in range(2):
                        ml = 2 * g + mi
                        m = 20 * rnd + ml
                        ps = ps1p.tile([128, 2, 512], f32, tag="f1")
                        for c in range(2):
                            for k in range(NKE):
                                nc.tensor.matmul(
                                    ps[:, c, :], w1t[:, mi, k],
                                    h1b[:, k, c * 512:(c + 1) * 512],
                                    start=(k == 0), stop=(k == NKE - 1))
                        nc.scalar.activation(
                            ffh[:, ml, :], ps.rearrange("p a b -> p (a b)"),
                            AF.Relu, bias=C("b1P")[:, m:m + 1])
            if True:
                for t in range(NT):
                    ts = slice(t * 128, (t + 1) * 128)
                    if rnd == 1:
                        out2 = o2a.tile([128, EMB], f32, tag="out2")
                        rsums = [ln2p.tile([128, 1], f32, tag=f"rs{n}",
                                           name=f"rs{n}") for n in range(3)]
                        accsum = ln2p.tile([128, 1], f32, tag="accsum",
                                           name="accsum")
                        nc.vector.reduce_sum(accsum[:], acc[:, t, :],
                                             axis=mybir.AxisListType.X)
                    psos = []
                    for n, (nb, nsz) in enumerate(NSL):
                        pso = ps2p.tile([128, nsz], f32, tag=f"pso{n}",
                                        name=f"pso{n}")
                        psos.append(pso)
                        nc.tensor.matmul(pso[:], ffh[:, 0, ts],
                                         w2t[:, 0, nb:nb + nsz],
                                         start=True, stop=False)
                        if rnd == 0:
                            _mmr(nc, pso[:], R("onesrow", 0, 128),
                                 R("b2row", nb, nb + nsz), False, False)
                            for kb in range(nb // 128, (nb + nsz) // 128):
                                nc.tensor.matmul(
                                    pso[:, kb * 128 - nb:kb * 128 - nb + 128],
                                    h1b[:, kb, ts], identh[:],
                                    start=False, stop=False)
                        for kl in range(1, 20):
                            nc.tensor.matmul(pso[:], ffh[:, kl, ts],
                                             w2t[:, kl, nb:nb + nsz],
                                             start=False, stop=(kl == 19))
                        if rnd == 0:
                            nc.scalar.activation(
                                acc[:, t, nb:nb + nsz], pso[:], AF.Identity)
                            continue
                        nc.scalar.activation(
                            out2[:, nb:nb + nsz], pso[:], AF.Identity,
                            accum_out=rsums[n][:])
                    if rnd == 0:
                        continue
                    # round 1: add the parked half, then LN2 + store
                    for n, (nb, nsz) in enumerate(NSL):
                        nc.vector.tensor_add(
                            out2[:, nb:nb + nsz], out2[:, nb:nb + nsz],
                            acc[:, t, nb:nb + nsz])
                    nmu = ln2p.tile([128, 1], f32, tag="nmu", name="nmu")
                    nc.vector.tensor_add(nmu[:], rsums[0][:], rsums[1][:])
                    nc.vector.tensor_add(nmu[:], nmu[:], rsums[2][:])
                    nc.vector.tensor_add(nmu[:], nmu[:], accsum[:])
                    nc.vector.tensor_scalar_mul(nmu[:], nmu[:], -1.0 / EMB)
                    cen = o2p.tile([128, EMB], f32, tag="cen")
                    nc.scalar.activation(cen[:], out2[:], AF.Identity, bias=nmu[:])
                    vv = ln2p.tile([128, 1], f32, tag="vv", name="vv")
                    sq2 = o2p.tile([128, EMB], f32, tag="sq2")
                    nc.scalar.activation(sq2[:], cen[:], AF.Square,
                                         accum_out=vv[:])
                    nc.vector.tensor_scalar_mul(vv[:], vv[:], 1.0 / EMB)
                    sdv = ln2p.tile([128, 1], f32, tag="sdv", name="sdv")
                    nc.scalar.activation(sdv[:], vv[:], AF.Sqrt, bias=epsP[:])
                    rv = ln2p.tile([128, 1], f32, tag="rv", name="rv")
                    with nc.allow_low_precision(reason="LN2 inv-std"):
                        nc.vector.reciprocal(rv[:], sdv[:])
                    nc.vector.tensor_mul(rv[:], rv[:], C("seqP")[:, t:t + 1])
                    eng = nc.vector if t % 2 == 1 else nc.gpsimd
                    t5 = o2p.tile([128, EMB], f32, tag="t5")
                    if t % 2 == 1:
                        eng.scalar_tensor_tensor(
                            t5[:], cen[:], rv[:], C("g2F"), OP.mult, OP.mult)
                    else:
                        # Pool cannot run STT on HW: scale on Act, mult on Pool
                        cen2 = o2p.tile([128, EMB], f32, tag="cen2")
                        nc.scalar.activation(cen2[:], cen[:], AF.Identity,
                                             scale=rv[:])
                        eng.tensor_mul(t5[:], cen2[:], C("g2F"))
                    eng.tensor_add(t5[:], t5[:], C("beta2F"))
                    nc.sync.dma_start(out_d[ts, :], t5[:])
        ffhp.release()
        accp.release()
        h1bp.release()
        constp.release()

    return nc


def _split_matmul_waits(bj: bytes) -> bytes:
    """Walrus codegen allows only one sync-wait on Matmult/DMACopy
    instructions; hoist extra waits onto a preceding EventSemaphore."""
    d = json.loads(bj)
    n = 0
    for f in d["functions"]:
        for blk in f["blocks"]:
            out = []
            for inst in blk["instructions"]:
                si = inst.get("sync_info")
                if (si and si.get("on_wait") and len(si["on_wait"]) >= 2
                        and inst.get("opcode") != "EventSemaphore"):
                    waits = si["on_wait"]
                    for w in waits[:-1]:
                        out.append({
                            "debug": inst.get("debug"),
                            "engine": inst["engine"],
                            "ins": [],
                            "outs": [],
                            "name": f"waitfix_{n}",
                            "opcode": "EventSemaphore",
                            "sync_info": {"on_update": [], "on_wait": [w]},
                        })
                        n += 1
                    si["on_wait"] = waits[-1:]
                out.append(inst)
            blk["instructions"] = out
    return json.dumps(d).encode()


_NC_CACHE = None


def _get_nc():
    global _NC_CACHE
    if _NC_CACHE is None:
        nc = build_nc()
        orig = nc.to_json_bytes
        nc.to_json_bytes = lambda: _split_matmul_waits(orig())
        _NC_CACHE = nc
    return _NC_CACHE


def _prep_core_inputs(x_b, mask_b, seq_b, conv_w, wq, bq, wk, bk, wv, bv, wo, bo,
                      w1, b1, w2, b2, g1, beta1, g2, beta2):
    f = np.float32
    x_b = np.asarray(x_b, dtype=f)                      # [T, EMB]
    xt = x_b.T.reshape(NKE, 128, T).transpose(1, 0, 2)  # [128, k, T]
    xh = x_b[:, CC:].T.reshape(NKD, 128, T).transpose(1, 0, 2)

    def wpack(w):  # [DM, DM] -> [g, p, mi, k, q]
        return np.ascontiguousarray(
            w.reshape(NKD, 128, 8, 128).transpose(2, 1, 0, 3)  # [m, p, k, q]
            .reshape(4, 2, 128, NKD, 128).transpose(0, 2, 1, 3, 4).astype(F16))

    wvh = np.ascontiguousarray(
        np.asarray(wv, f).reshape(NKD, 128, DM).transpose(1, 0, 2).astype(F16))
    w1h = np.ascontiguousarray(
        np.asarray(w1, f).reshape(NKE, 128, 40, 128)
        .transpose(2, 1, 0, 3)                       # [m, p, k, q]
        .reshape(20, 2, 128, NKE, 128).transpose(0, 2, 1, 3, 4).astype(F16))
    w2h = np.ascontiguousarray(
        np.asarray(w2, f).reshape(2, 20, 128, EMB).transpose(0, 2, 1, 3).astype(F16))

    # reference: scores = where(att_mask != 0, -1e9, scores) — attended
    # keys are those with att_mask == 0
    maskf = (np.asarray(mask_b) == 0).astype(f)
    consts = np.zeros((128, NCONST), f)

    def setC(name, val):
        a, b = _C[name]
        consts[:, a:b] = val

    setC("bqP", np.asarray(bq, f).reshape(8, 128).T)
    setC("bkP", np.asarray(bk, f).reshape(8, 128).T)
    setC("boP", np.asarray(bo, f).reshape(8, 128).T)
    setC("maskP", maskf.reshape(8, 128).T)
    setC("seqP", np.asarray(seq_b, f).reshape(8, 128).T)
    setC("b1P", np.asarray(b1, f).reshape(40, 128).T)
    setC("g1P", np.asarray(g1, f).reshape(10, 128).T)
    setC("beta1P", np.asarray(beta1, f).reshape(10, 128).T)
    setC("g2F", np.tile(np.asarray(g2, f)[None, :], (128, 1)))
    setC("beta2F", np.tile(np.asarray(beta2, f)[None, :], (128, 1)))
    setC("onescol", 1.0)

    crow = np.zeros((1, NROW), f)

    def setR(name, val):
        a, b = _R[name]
        crow[0, a:b] = val

    setR("onesrow", 1.0)
    setR("bvrow", np.asarray(bv, f))
    setR("b2row", np.asarray(b2, f))
    setR("seqrow", np.asarray(seq_b, f))

    return {
        "xt": np.ascontiguousarray(xt),
        "xh": np.ascontiguousarray(xh.astype(F16)),
        "wqh": wpack(np.asarray(wq, f)),
        "wkh": wpack(np.asarray(wk, f)),
        "wvh": wvh,
        "woh": wpack(np.asarray(wo, f)),
        "w1h": w1h,
        "w2h": w2h,
        "consts": consts,
        "crow": crow,
        "identh": np.eye(128).astype(F16),
        "cwbc": np.tile(np.asarray(conv_w, f).reshape(K)[None, :], (128, 1)),
        "onescol": np.ones((128, 1), f),
    }


def kernel(x, att_mask, seq_mask, conv_w, wq, bq, wk, bk, wv, bv, wo, bo,
           w1, b1, w2, b2, g1, beta1, g2, beta2, _trace=False):
    from concourse.bass_utils import run_bass_kernel_spmd

    nc = _get_nc()
    x = np.asarray(x, dtype=np.float32)
    in_maps = []
    for b in range(B):
        in_maps.append(_prep_core_inputs(
            x[b], np.asarray(att_mask)[b], np.asarray(seq_mask)[b, :, 0],
            np.asarray(conv_w), np.asarray(wq), np.asarray(bq), np.asarray(wk),
            np.asarray(bk), np.asarray(wv), np.asarray(bv), np.asarray(wo),
            np.asarray(bo), np.asarray(w1), np.asarray(b1), np.asarray(w2),
            np.asarray(b2), np.asarray(g1), np.asarray(beta1), np.asarray(g2),
            np.asarray(beta2)))
    res = run_bass_kernel_spmd(nc, in_maps, list(range(B)), trace=_trace)
    out = np.stack([res.results[i]["out"] for i in range(B)], axis=0)
    if _trace:
        return out, res
    return out


# revision 6
# speedup vs baseline: 1.0099x; 1.0035x over previous
"""Trainium2 Bass/Tile kernel for nn_EncoderLayer (dense transformer block).

Data-parallel over batch (B=8 -> 1 element/core, no collectives). v2:
- All matmuls in fp16 (full PE rate, ~0.02% rel err) with f32 PSUM; LN
  stats and residual paths in f32.
- Attention: feature-major Q/K/V; softmax via exp(s/8 - 3) with the shift
  cancelling against an appended mask-column normalizer; the 1/norm
  broadcast is rebuilt inside the retired PV accumulator bank so attention
  fits exactly in 8 PSUM banks; normalized heads are written back into the
  dead kt slices (attT aliases kt's storage).
- LN1 stays in SBUF; biases/residuals fold into PSUM seeds (rank-1
  ones-outer-products) or STT evictions; conv branch runs in-place on
  Pool/DVE.
- FFN runs in two k-rounds (FFN1 half -> FFN2 half) so the ff activations
  and the w2 half fit SBUF together; round 0 seeds b2 + the h1 residual
  (fp16 identity-matmul transposes) and parks partials in fp16; round 1
  adds the second half and runs LN2 per time tile, streaming out.
"""

import json
import sys

if "/opt/trn_rl_repo" not in sys.path:
    sys.path.insert(0, "/opt/trn_rl_repo")

import numpy as np

import concourse.bass as bass
import concourse.mybir as mybir
import concourse.tile as tile

B, T, CC, DM, H, DH, DFF, K = 8, 1024, 256, 1024, 16, 64, 5120, 3
EMB = CC + DM  # 1280
EPS = 1e-6
f32 = mybir.dt.float32
f32r = mybir.dt.float32r
f16 = mybir.dt.float16
AF = mybir.ActivationFunctionType
OP = mybir.AluOpType

NT = T // 128          # 8 time tiles
NKE = EMB // 128       # 10 embed k-tiles
NKD = DM // 128        # 8 d_model k-tiles
HV = DH + 1            # 65 = V dims + mask column
F16 = np.float16

# column offsets in the packed [128, NCONST] f32 const blob
_C = {}
_o = 0
for _name, _w in (("bqP", 8), ("bkP", 8), ("boP", 8), ("maskP", 8),
                  ("seqP", 8), ("b1P", 40), ("g1P", 10), ("beta1P", 10),
                  ("g2F", EMB), ("beta2F", EMB), ("onescol", 1)):
    _C[_name] = (_o, _o + _w)
    _o += _w
NCONST = _o
# row-vector blob [1, NROW]
_R = {}
_o = 0
for _name, _w in (("onesrow", 128), ("bvrow", DM), ("b2row", EMB),
                  ("seqrow", T)):
    _R[_name] = (_o, _o + _w)
    _o += _w
NROW = _o


def _mmr(nc, out, lhsT, rhs, start, stop):
    nc.tensor.matmul(out, lhsT.bitcast(f32r), rhs.bitcast(f32r),
                     start=start, stop=stop)


def build_nc(phase=99):
    import os
    phase = int(os.environ.get("KPHASE", phase))
    nc = bass.Bass()

    xt_d = nc.declare_dram_parameter("xt", [128, NKE, T], f32, isOutput=False)
    xh_d = nc.declare_dram_parameter("xh", [128, NKD, T], f16, isOutput=False)
    wqh_d = nc.declare_dram_parameter("wqh", [4, 128, 2, NKD, 128], f16, isOutput=False)
    wkh_d = nc.declare_dram_parameter("wkh", [4, 128, 2, NKD, 128], f16, isOutput=False)
    wvh_d = nc.declare_dram_parameter("wvh", [128, NKD, DM], f16, isOutput=False)
    woh_d = nc.declare_dram_parameter("woh", [4, 128, 2, NKD, 128], f16, isOutput=False)
    w1h_d = nc.declare_dram_parameter("w1h", [20, 128, 2, NKE, 128], f16, isOutput=False)
    w2h_d = nc.declare_dram_parameter("w2h", [2, 128, 20, EMB], f16, isOutput=False)
    consts_d = nc.declare_dram_parameter("consts", [128, NCONST], f32, isOutput=False)
    crow_d = nc.declare_dram_parameter("crow", [1, NROW], f32, isOutput=False)
    identh_d = nc.declare_dram_parameter("identh", [128, 128], f16, isOutput=False)
    cwbc_d = nc.declare_dram_parameter("cwbc", [128, K], f32, isOutput=False)
    onescol_d = nc.declare_dram_parameter("onescol", [128, 1], f32, isOutput=False)
    out_d = nc.declare_dram_parameter("out", [T, EMB], f32, isOutput=True)

    with tile.TileContext(nc) as tc:
        constp = tc.alloc_tile_pool(name="constp", bufs=1)
        cb = constp.tile([128, NCONST], f32)
        cr = constp.tile([1, NROW], f32r)
        identh = constp.tile([128, 128], f16)
        cwbc = constp.tile([128, K], f32)
        epsP = constp.tile([128, 1], f32)
        nc.gpsimd.memset(epsP[:], EPS)
        nthreeP = constp.tile([128, 1], f32)
        nc.gpsimd.memset(nthreeP[:], -3.0)
        onescolP = constp.tile([128, 1], f32r)

        def C(name):
            a, b = _C[name]
            return cb[:, a:b]

        def R(name, lo=None, hi=None):
            a, b = _R[name]
            if lo is not None:
                return cr[:, a + lo:a + hi]
            return cr[:, a:b]

        # ------------- persistent pools (right stack, LIFO by release) ----
        h1prep = tc.alloc_tile_pool(name="h1prep", bufs=1, side="right")
        h1pre = h1prep.tile([128, NKE, T], f32r)
        xtp = tc.alloc_tile_pool(name="xtp", bufs=1, side="right")
        xt = xtp.tile([128, NKE, T], f32)
        qktp = tc.alloc_tile_pool(name="qktp", bufs=1, side="right")
        qt = qktp.tile([128, NKD, T], f16)
        kt = qktp.tile([128, NKD, T], f16)   # attT aliases kt after scores
        vaup = tc.alloc_tile_pool(name="vaup", bufs=1, side="right")
        vaug = vaup.tile([128, NT, H, HV], f16)
        wvp = tc.alloc_tile_pool(name="wvp", bufs=1, side="right")
        wvh = wvp.tile([128, NKD, DM], f16)
        xhp = tc.alloc_tile_pool(name="xhp", bufs=1, side="right")
        xh = xhp.tile([128, NKD, T], f16)

        # xh + the first projection weights gate the PE pipeline; everything
        # else (consts, xt, wv) rides the Act queue or follows on SP.
        nc.sync.dma_start(xh[:, 0:4, :], xh_d[:, 0:4, :])

        # ------------- Q/K/V projections (fp16) ---------------------------
        with (
            tc.tile_pool(name="wst", bufs=4) as wst,
            tc.tile_pool(name="qkps", bufs=2, space="PSUM") as qkps,
            tc.tile_pool(name="vps", bufs=2, space="PSUM") as vps,
        ):
            # pre-issue the wq stream on SP (it gates the PE pipeline); the
            # big non-urgent loads follow on SP; wk groups are issued on the
            # Act queue as their slot frees up during the Q pass
            wts = []
            for g in range(4):
                wt = wst.tile([128, 2, NKD, 128], f16, tag="w")
                nc.sync.dma_start(wt[:], wqh_d[g])
                wts.append(wt)
                if g == 0:
                    nc.sync.dma_start(xh[:, 4:NKD, :], xh_d[:, 4:NKD, :])
            nc.sync.dma_start(cb[:, 0:100], consts_d[:, 0:100])
            nc.sync.dma_start(cr[:], crow_d[:].bitcast(f32r))
            nc.sync.dma_start(cb[:, 100:NCONST], consts_d[:, 100:NCONST])
            nc.sync.dma_start(xt[:, 0:2, :], xt_d[:, 0:2, :])
            nc.sync.dma_start(cwbc[:], cwbc_d[:])
            nc.sync.dma_start(onescolP[:], onescol_d[:].bitcast(f32r))
            nc.sync.dma_start(xt[:, 2:NKE, :], xt_d[:, 2:NKE, :])
            nc.sync.dma_start(wvh[:], wvh_d[:])
            nc.sync.dma_start(identh[:], identh_d[:])

            # conv branch (in-place in h1pre, Pool/DVE):
            # y[t] = w0*x[t-1] + w1*x[t] + w2*x[t+1], zero-padded; then +x.
            for kb, eng in ((0, nc.vector), (1, nc.vector)):
                dst = h1pre[:, kb, :]
                eng.tensor_scalar_mul(dst, xt[:, kb, :], cwbc[:, 1:2])
                eng.scalar_tensor_tensor(
                    dst[:, 0:T - 1], xt[:, kb, 1:T], cwbc[:, 2:3],
                    dst[:, 0:T - 1], OP.mult, OP.add)
                eng.scalar_tensor_tensor(
                    dst[:, 1:T], xt[:, kb, 0:T - 1], cwbc[:, 0:1],
                    dst[:, 1:T], OP.mult, OP.add)
                eng.tensor_add(dst, dst, xt[:, kb, :])

            def proj_group(wt, dst, bias, g):
                for mi in range(2):
                    m = 2 * g + mi
                    ps = qkps.tile([128, 2, 512], f32, tag="qk")
                    for c in range(2):
                        for k in range(NKD):
                            nc.tensor.matmul(
                                ps[:, c, :], wt[:, mi, k],
                                xh[:, k, c * 512:(c + 1) * 512],
                                start=(k == 0), stop=(k == NKD - 1))
                    nc.scalar.activation(
                        dst[:, m, :], ps.rearrange("p a b -> p (a b)"),
                        AF.Identity, bias=C(bias)[:, m:m + 1])

            wkts = []
            for g in range(4):
                proj_group(wts[g], qt, "bqP", g)
                wkt = wst.tile([128, 2, NKD, 128], f16, tag="w")
                nc.scalar.dma_start(wkt[:], wkh_d[g])
                wkts.append(wkt)
            for g in range(4):
                proj_group(wkts[g], kt, "bkP", g)
            for i in range(NT):
                for n in range(2):
                    ps = vps.tile([128, 512], f32, tag="v")
                    _mmr(nc, ps[:], R("onesrow", 0, 128),
                         R("bvrow", n * 512, (n + 1) * 512), True, False)
                    for k in range(NKD):
                        nc.tensor.matmul(
                            ps[:], xh[:, k, i * 128:(i + 1) * 128],
                            wvh[:, k, n * 512:(n + 1) * 512],
                            start=False, stop=(k == NKD - 1))
                    dest = vaug[:, i, n * 8:(n + 1) * 8, 0:DH]
                    nc.vector.tensor_scalar_mul(
                        dest, ps.rearrange("p (h c) -> p h c", c=DH),
                        C("maskP")[:, i:i + 1])
                mcols = vaug[:, i, :, DH:DH + 1].rearrange("p h c -> p (h c)")
                nc.vector.tensor_copy(
                    mcols, C("maskP")[:, i:i + 1].to_broadcast([128, H]))
        xhp.release()
        wvp.release()

        # ------------- attention ------------------------------------------
        if phase >= 2:
            with (
                tc.tile_pool(name="spsp", bufs=2, space="PSUM") as spsp,
                tc.tile_pool(name="apsp", bufs=2, space="PSUM") as apsp,
                tc.tile_pool(name="u2p", bufs=3) as u2p,
                tc.tile_pool(name="finp", bufs=1) as finp,
            ):
                for h in range(H):
                    ktile, prow = h // 2, (h % 2) * 64
                    aps = apsp.tile([HV, 2, 512], f32, tag="aps")
                    for jt in range(NT):
                        sps = spsp.tile([128, 2, 512], f32, tag="sps")
                        for c in range(2):
                            nc.tensor.matmul(
                                sps[:, c, :],
                                kt[prow:prow + 64, ktile, jt * 128:(jt + 1) * 128],
                                qt[prow:prow + 64, ktile, c * 512:(c + 1) * 512],
                                start=True, stop=True)
                        u2t = u2p.tile([128, T], f16, tag="u2")
                        # exp(s/8 - 3): the shift cancels against the
                        # mask-column normalizer; keeps u in fp16 range
                        nc.scalar.activation(
                            u2t[:], sps.rearrange("p a b -> p (a b)"),
                            AF.Exp, scale=0.125, bias=nthreeP[:])
                        for c in range(2):
                            nc.tensor.matmul(
                                aps[:, c, :], vaug[:, jt, h, :],
                                u2t[:, c * 512:(c + 1) * 512],
                                start=(jt == 0), stop=(jt == NT - 1))
                    # finalize: 1/norm, copy out the unnormalized head,
                    # broadcast 1/norm into the retired aps bank, then
                    # scale-evict into the dead kt slice (attT alias)
                    nt_ = finp.tile([1, T], f32r, tag=f"nt{h % 2}", name=f"nt{h % 2}")
                    with nc.allow_low_precision(reason="softmax normalizer"):
                        nc.vector.reciprocal(
                            nt_[:], aps[DH:HV, :, :].rearrange("p a b -> p (a b)"))
                    ab = finp.tile([64, T], f16, tag=f"ab{h % 2}", name=f"ab{h % 2}")
                    nc.vector.tensor_copy(
                        ab[:], aps[0:DH, :, :].rearrange("p a b -> p (a b)"))
                    for c in range(2):
                        _mmr(nc, aps[0:DH, c, :], R("onesrow", 0, DH),
                             nt_[:, c * 512:(c + 1) * 512], True, True)
                    nc.vector.tensor_mul(
                        kt[prow:prow + 64, ktile, :], ab[:],
                        aps[0:DH, :, :].rearrange("p a b -> p (a b)"))
            vaup.release()

        # ------------- out-proj + LN1 -------------------------------------
        if phase >= 3:
            h1bp = tc.alloc_tile_pool(name="h1bp", bufs=1)
            h1b = h1bp.tile([128, NKE, T], f16)
            # preload the sqrt activation table off the critical path (the
            # attention exps are done; everything later lives in the
            # sqrt_and_others table)
            scr1 = h1bp.tile([128, 1], f32)
            nc.scalar.activation(scr1[:], epsP[:], AF.Sqrt, bias=epsP[:])
            with (
                tc.tile_pool(name="wost", bufs=3) as wost,
                tc.tile_pool(name="ops", bufs=4, space="PSUM") as opsp,
                tc.tile_pool(name="lnps", bufs=1, space="PSUM") as lnps,
                tc.tile_pool(name="sqp", bufs=2) as sqp,
            ):
                musum = lnps.tile([1, 2, 512], f32, tag="musum")
                sqsum = lnps.tile([1, 2, 512], f32, tag="sqsum")

                def stats(k):
                    for c in range(2):
                        cs = slice(c * 512, (c + 1) * 512)
                        sq = sqp.tile([128, 512], f32r, tag="sq")
                        nc.vector.tensor_mul(sq[:], h1pre[:, k, cs], h1pre[:, k, cs])
                        _mmr(nc, musum[:, c, :], onescolP[:],
                             h1pre[:, k, cs], k == 0, k == NKE - 1)
                        _mmr(nc, sqsum[:, c, :], onescolP[:],
                             sq[:], k == 0, k == NKE - 1)

                stats(0)
                stats(1)
                for g in range(4):
                    wt = wost.tile([128, 2, NKD, 128], f16, tag="wo")
                    nc.scalar.dma_start(wt[:], woh_d[g])
                    for mi in range(2):
                        m = 2 * g + mi
                        for c in range(2):
                            cs = slice(c * 512, (c + 1) * 512)
                            ps = opsp.tile([128, 512], f32, tag="o")
                            for k in range(NKD):
                                nc.tensor.matmul(
                                    ps[:], wt[:, mi, k], kt[:, k, cs],
                                    start=(k == 0), stop=(k == NKD - 1))
                            nc.vector.scalar_tensor_tensor(
                                h1pre[:, 2 + m, cs], ps[:],
                                C("boP")[:, m:m + 1],
                                xt[:, 2 + m, cs], OP.add, OP.add)
                            del ps
                        # stats lag one m-tile so the PE never waits on the
                        # DVE eviction of the tile it is summing
                        if m >= 1:
                            stats(1 + m)
                stats(9)

                # ---------------- LN1 scalars + broadcasts ----------------
                with tc.tile_pool(name="lnvp", bufs=1) as lnvp:
                    mu = lnvp.tile([1, T], f32r)
                    nc.vector.tensor_scalar_mul(
                        mu[:], musum.rearrange("p a b -> p (a b)"), 1.0 / EMB)
                    ex2 = lnvp.tile([1, T], f32r)
                    nc.vector.tensor_scalar_mul(
                        ex2[:], sqsum.rearrange("p a b -> p (a b)"), 1.0 / EMB)
                    sd = lnvp.tile([1, T], f32r)
                    nc.vector.tensor_mul(sd[:], mu[:], mu[:])
                    nc.vector.tensor_sub(ex2[:], ex2[:], sd[:])
                    nc.scalar.activation(sd[:], ex2[:], AF.Sqrt, bias=epsP[0:1, :])
                    rs = ex2  # reuse (dead after the Sqrt read)
                    with nc.allow_low_precision(reason="LN1 inv-std"):
                        nc.vector.reciprocal(rs[:], sd[:])
                    nc.vector.tensor_mul(rs[:], rs[:], R("seqrow"))
                    muF = lnvp.tile([128, T], f16)
                    rsF = lnvp.tile([128, T], f16)
                    for c in range(2):
                        cs = slice(c * 512, (c + 1) * 512)
                        pb = opsp.tile([128, 512], f32, tag="o")
                        _mmr(nc, pb[:], R("onesrow", 0, 128), mu[:, cs], True, True)
                        nc.scalar.activation(muF[:, cs], pb[:], AF.Copy)
                        pb2 = opsp.tile([128, 512], f32, tag="o")
                        _mmr(nc, pb2[:], R("onesrow", 0, 128), rs[:, cs], True, True)
                        nc.scalar.activation(rsF[:, cs], pb2[:], AF.Copy)

                    # ------------- LN1 normalize, c-half major -----------
                    for c in range(2):
                        cs = slice(c * 512, (c + 1) * 512)
                        for k in range(NKE):
                            eng = nc.vector if k % 2 == 0 else nc.gpsimd
                            t1 = sqp.tile([128, 512], f32, tag=f"t1{k % 2}",
                                          name=f"t1{k % 2}")
                            eng.tensor_sub(t1[:], h1pre[:, k, cs], muF[:, cs])
                            t2 = sqp.tile([128, 512], f32, tag=f"t2{k % 2}",
                                          name=f"t2{k % 2}")
                            eng.tensor_mul(t2[:], t1[:], rsF[:, cs])
                            nc.scalar.activation(
                                h1b[:, k, cs], t2[:], AF.Identity,
                                bias=C("beta1P")[:, k:k + 1],
                                scale=C("g1P")[:, k:k + 1])

        if phase < 4:
            with tc.tile_pool(name="dummy", bufs=1) as dum:
                z = dum.tile([128, EMB], f32)
                nc.gpsimd.memset(z[:], 0.0)
                for t in range(NT):
                    nc.sync.dma_start(out_d[t * 128:(t + 1) * 128, :], z[:])
            constp.release()
            return nc

        qktp.release()
        xtp.release()
        h1prep.release()

        # ------------- FFN in two k-rounds + LN2 --------------------------
        accp = tc.alloc_tile_pool(name="accp", bufs=1)
        acc = accp.tile([128, NT, EMB], f16)
        ffhp = tc.alloc_tile_pool(name="ffhp", bufs=1)
        NSL = ((0, 512), (512, 512), (1024, 256))
        # w1st allocated below w2hp so the streamed w1 tiles do not overlap
        # the (still-live) h1pre region and get WAR-gated behind LN1
        with (
            tc.tile_pool(name="w1st", bufs=3) as w1st,
            tc.tile_pool(name="w2hp", bufs=1) as w2hp,
            tc.tile_pool(name="ps1", bufs=2, space="PSUM") as ps1p,
            tc.tile_pool(name="ps2", bufs=1, space="PSUM") as ps2p,
            tc.tile_pool(name="o2a", bufs=2) as o2a,
            tc.tile_pool(name="o2p", bufs=1) as o2p,
            tc.tile_pool(name="ln2p", bufs=2) as ln2p,
        ):
          for rnd in range(2):
            ffh = ffhp.tile([128, 20, T], f16, tag="ffh")
            w2t = w2hp.tile([128, 20, EMB], f16, tag="w2t")
            if True:
                for g in range(10):
                    w1t = w1st.tile([128, 2, NKE, 128], f16, tag="w1")
                    nc.sync.dma_start(w1t[:], w1h_d[10 * rnd + g])
                    if g == 2:
                        # w2 half in chunks behind the first w1 tiles: keeps
                        # the DMA pipe busy without head-of-line blocking
                        for cch in range(4):
                            nc.sync.dma_start(
                                w2t[:, 5 * cch:5 * cch + 5, :],
                                w2h_d[rnd, :, 5 * cch:5 * cch + 5, :])
                    for mi in range(2):
                        ml = 2 * g + mi
                        m = 20 * rnd + ml
                        ps = ps1p.tile([128, 2, 512], f32, tag="f1")
                        for c in range(2):
                            for k in range(NKE):
                                nc.tensor.matmul(
                                    ps[:, c, :], w1t[:, mi, k],
                                    h1b[:, k, c * 512:(c + 1) * 512],
                                    start=(k == 0), stop=(k == NKE - 1))
                        nc.scalar.activation(
                            ffh[:, ml, :], ps.rearrange("p a b -> p (a b)"),
                            AF.Relu, bias=C("b1P")[:, m:m + 1])
            if True:
                for t in range(NT):
                    ts = slice(t * 128, (t + 1) * 128)
                    if rnd == 1:
                        out2 = o2a.tile([128, EMB], f32, tag="out2")
                        rsums = [ln2p.tile([128, 1], f32, tag=f"rs{n}",
                                           name=f"rs{n}") for n in range(3)]
                        accsum = ln2p.tile([128, 1], f32, tag="accsum",
                                           name="accsum")
                        nc.vector.reduce_sum(accsum[:], acc[:, t, :],
                                             axis=mybir.AxisListType.X)
                    psos = []
                    for n, (nb, nsz) in enumerate(NSL):
                        pso = ps2p.tile([128, nsz], f32, tag=f"pso{n}",
                                        name=f"pso{n}")
                        psos.append(pso)
                        nc.tensor.matmul(pso[:], ffh[:, 0, ts],
                                         w2t[:, 0, nb:nb + nsz],
                                         start=True, stop=False)
                        if rnd == 0:
                            _mmr(nc, pso[:], R("onesrow", 0, 128),
                                 R("b2row", nb, nb + nsz), False, False)
                            for kb in range(nb // 128, (nb + nsz) // 128):
                                nc.tensor.matmul(
                                    pso[:, kb * 128 - nb:kb * 128 - nb + 128],
                                    h1b[:, kb, ts], identh[:],
                                    start=False, stop=False)
                        for kl in range(1, 20):
                            nc.tensor.matmul(pso[:], ffh[:, kl, ts],
                                             w2t[:, kl, nb:nb + nsz],
                                             start=False, stop=(kl == 19))
                        if rnd == 0:
                            nc.scalar.activation(
                                acc[:, t, nb:nb + nsz], pso[:], AF.Identity)
                            continue
                        nc.scalar.activation(
                            out2[:, nb:nb + nsz], pso[:], AF.Identity,
                            accum_out=rsums[n][:])
                    if rnd == 0:
                        continue
                    # round 1: add the parked half, then LN2 + store
                    for n, (nb, nsz) in enumerate(NSL):
                        nc.vector.tensor_add(
                            out2[:, nb:nb + nsz], out2[:, nb:nb + nsz],
                            acc[:, t, nb:nb + nsz])
                    nmu = ln2p.tile([128, 1], f32, tag="nmu", name="nmu")
                    nc.vector.tensor_add(nmu[:], rsums[0][:], rsums[1][:])
                    nc.vector.tensor_add(nmu[:], nmu[:], rsums[2][:])
                    nc.vector.tensor_add(nmu[:], nmu[:], accsum[:])
                    nc.vector.tensor_scalar_mul(nmu[:], nmu[:], -1.0 / EMB)
                    # y = (out2 - mu) * g2 runs in parallel with the variance
                    # chain; final = y * rv + beta2 (scalar reassociation)
                    y = o2p.tile([128, EMB], f32, tag="t5")
                    nc.vector.scalar_tensor_tensor(
                        y[:], out2[:], nmu[:], C("g2F"), OP.add, OP.mult)
                    # var = E[x^2] - mu^2 via Act Square row-accumulate
                    vv = ln2p.tile([128, 1], f32, tag="vv", name="vv")
                    sq2 = o2p.tile([128, EMB], f32, tag="sq2")
                    nc.scalar.activation(sq2[:], out2[:], AF.Square,
                                         accum_out=vv[:])
                    nc.vector.tensor_scalar_mul(vv[:], vv[:], 1.0 / EMB)
                    mumu = ln2p.tile([128, 1], f32, tag="mumu", name="mumu")
                    nc.vector.tensor_mul(mumu[:], nmu[:], nmu[:])
                    nc.vector.tensor_sub(vv[:], vv[:], mumu[:])
                    sdv = ln2p.tile([128, 1], f32, tag="sdv", name="sdv")
                    nc.scalar.activation(sdv[:], vv[:], AF.Sqrt, bias=epsP[:])
                    rv = ln2p.tile([128, 1], f32, tag="rv", name="rv")
                    with nc.allow_low_precision(reason="LN2 inv-std"):
                        nc.vector.reciprocal(rv[:], sdv[:])
                    nc.vector.tensor_mul(rv[:], rv[:], C("seqP")[:, t:t + 1])
                    t6 = o2p.tile([128, EMB], f32, tag="cen")
                    nc.vector.scalar_tensor_tensor(
                        t6[:], y[:], rv[:], C("beta2F"), OP.mult, OP.add)
                    nc.sync.dma_start(out_d[ts, :], t6[:])
        ffhp.release()
        accp.release()
        h1bp.release()
        constp.release()

    return nc


def _split_matmul_waits(bj: bytes) -> bytes:
    """Walrus codegen allows only one sync-wait on Matmult/DMACopy
    instructions; hoist extra waits onto a preceding EventSemaphore."""
    d = json.loads(bj)
    n = 0
    for f in d["functions"]:
        for blk in f["blocks"]:
            out = []
            for inst in blk["instructions"]:
                si = inst.get("sync_info")
                if (si and si.get("on_wait") and len(si["on_wait"]) >= 2
                        and inst.get("opcode") != "EventSemaphore"):
                    waits = si["on_wait"]
                    for w in waits[:-1]:
                        out.append({
                            "debug": inst.get("debug"),
                            "engine": inst["engine"],
                            "ins": [],
                            "outs": [],
                            "name": f"waitfix_{n}",
                            "opcode": "EventSemaphore",
                            "sync_info": {"on_update": [], "on_wait": [w]},
                        })
                        n += 1
                    si["on_wait"] = waits[-1:]
                out.append(inst)
            blk["instructions"] = out
    return json.dumps(d).encode()


_NC_CACHE = None


def _get_nc():
    global _NC_CACHE
    if _NC_CACHE is None:
        nc = build_nc()
        orig = nc.to_json_bytes
        nc.to_json_bytes = lambda: _split_matmul_waits(orig())
        _NC_CACHE = nc
    return _NC_CACHE


def _prep_core_inputs(x_b, mask_b, seq_b, conv_w, wq, bq, wk, bk, wv, bv, wo, bo,
                      w1, b1, w2, b2, g1, beta1, g2, beta2):
    f = np.float32
    x_b = np.asarray(x_b, dtype=f)                      # [T, EMB]
    xt = x_b.T.reshape(NKE, 128, T).transpose(1, 0, 2)  # [128, k, T]
    xh = x_b[:, CC:].T.reshape(NKD, 128, T).transpose(1, 0, 2)

    def wpack(w):  # [DM, DM] -> [g, p, mi, k, q]
        return np.ascontiguousarray(
            w.reshape(NKD, 128, 8, 128).transpose(2, 1, 0, 3)  # [m, p, k, q]
            .reshape(4, 2, 128, NKD, 128).transpose(0, 2, 1, 3, 4).astype(F16))

    wvh = np.ascontiguousarray(
        np.asarray(wv, f).reshape(NKD, 128, DM).transpose(1, 0, 2).astype(F16))
    w1h = np.ascontiguousarray(
        np.asarray(w1, f).reshape(NKE, 128, 40, 128)
        .transpose(2, 1, 0, 3)                       # [m, p, k, q]
        .reshape(20, 2, 128, NKE, 128).transpose(0, 2, 1, 3, 4).astype(F16))
    w2h = np.ascontiguousarray(
        np.asarray(w2, f).reshape(2, 20, 128, EMB).transpose(0, 2, 1, 3).astype(F16))

    # reference: scores = where(att_mask != 0, -1e9, scores) — attended
    # keys are those with att_mask == 0
    maskf = (np.asarray(mask_b) == 0).astype(f)
    consts = np.zeros((128, NCONST), f)

    def setC(name, val):
        a, b = _C[name]
        consts[:, a:b] = val

    setC("bqP", np.asarray(bq, f).reshape(8, 128).T)
    setC("bkP", np.asarray(bk, f).reshape(8, 128).T)
    setC("boP", np.asarray(bo, f).reshape(8, 128).T)
    setC("maskP", maskf.reshape(8, 128).T)
    setC("seqP", np.asarray(seq_b, f).reshape(8, 128).T)
    setC("b1P", np.asarray(b1, f).reshape(40, 128).T)
    setC("g1P", np.asarray(g1, f).reshape(10, 128).T)
    setC("beta1P", np.asarray(beta1, f).reshape(10, 128).T)
    setC("g2F", np.tile(np.asarray(g2, f)[None, :], (128, 1)))
    setC("beta2F", np.tile(np.asarray(beta2, f)[None, :], (128, 1)))
    setC("onescol", 1.0)

    crow = np.zeros((1, NROW), f)

    def setR(name, val):
        a, b = _R[name]
        crow[0, a:b] = val

    setR("onesrow", 1.0)
    setR("bvrow", np.asarray(bv, f))
    setR("b2row", np.asarray(b2, f))
    setR("seqrow", np.asarray(seq_b, f))

    return {
        "xt": np.ascontiguousarray(xt),
        "xh": np.ascontiguousarray(xh.astype(F16)),
        "wqh": wpack(np.asarray(wq, f)),
        "wkh": wpack(np.asarray(wk, f)),
        "wvh": wvh,
        "woh": wpack(np.asarray(wo, f)),
        "w1h": w1h,
        "w2h": w2h,
        "consts": consts,
        "crow": crow,
        "identh": np.eye(128).astype(F16),
        "cwbc": np.tile(np.asarray(conv_w, f).reshape(K)[None, :], (128, 1)),
        "onescol": np.ones((128, 1), f),
    }


def kernel(x, att_mask, seq_mask, conv_w, wq, bq, wk, bk, wv, bv, wo, bo,
           w1, b1, w2, b2, g1, beta1, g2, beta2, _trace=False):
    from concourse.bass_utils import run_bass_kernel_spmd

    nc = _get_nc()
    x = np.asarray(x, dtype=np.float32)
    in_maps = []
    for b in range(B):
        in_maps.append(_prep_core_inputs(
            x[b], np.asarray(att_mask)[b], np.asarray(seq_mask)[b, :, 0],
            np.asarray(conv_w), np.asarray(wq), np.asarray(bq), np.asarray(wk),
            np.asarray(bk), np.asarray(wv), np.asarray(bv), np.asarray(wo),
            np.asarray(bo), np.asarray(w1), np.asarray(b1), np.asarray(w2),
            np.asarray(b2), np.asarray(g1), np.asarray(beta1), np.asarray(g2),
            np.asarray(beta2)))
    res = run_bass_kernel_spmd(nc, in_maps, list(range(B)), trace=_trace)
    out = np.stack([res.results[i]["out"] for i in range(B)], axis=0)
    if _trace:
        return out, res
    return out
